# revision 1
# baseline (speedup 1.0000x reference)
"""DCT-compressed attention (nn_DCTAttentionIdeal) on 8 Trainium2 NeuronCores.

Math (per head, reference ordering):
    S    = (Q*s) @ (K*mask*s)^T with s = D**-0.25             [N,N]
    atn  = softmax(S, axis=-1)
    Vd   = Qd @ (V*mask)                                      [M,D]
    out  = Qd^T @ ((Qd @ atn @ Qd^T) @ Vd)                    [N,D]

Kernel reshaping (exact in real arithmetic):
  - softmax max-subtraction skipped (scores ~N(0,1) after the 1/8 scale,
    which is folded into the Exp activation's `scale`).
  - per-row 1/denom folded into DCT columns:
        A1^T[k,m] = sum_q exp(S)[q,k] * (Qd^T[q,m]/denom[q])
    so the [N,N] exp matrix is consumed unnormalized straight from SBUF.
  - final contraction reassociated: out = Qd^T @ (G @ Vd), G = A1 @ Qd^T.

dtypes: the two O(N^2 M) matmuls (scores' exp -> A1^T) run bf16 (exp storage);
everything else runs float32r (tf32-class precision, bf16-class speed).
Phase A (exp on ScalarE) of one q-group overlaps phase B (A1^T on TensorE)
of the previous group via a 2-group software pipeline; GT accumulates both
groups' partial A1^T tiles so no extra adds are needed.

Sharding: batch*heads (2*16=32) split 4-per-core across 8 cores; Q_dct
replicated; no cross-core communication.  Host pre-transposes Q and Q_dct
(pure layout); masking, K transpose, softmax and all DCT algebra run
on-device.
"""

import numpy as np
import ml_dtypes

import concourse.tile as tile
from concourse import bacc, mybir
from concourse import bass_utils

F32 = mybir.dt.float32
BF16 = mybir.dt.bfloat16
F32R = mybir.dt.float32r
NPBF16 = ml_dtypes.bfloat16
AF = mybir.ActivationFunctionType
ALU = mybir.AluOpType
AX = mybir.AxisListType

B, H, N, D, M = 2, 16, 2048, 64, 256
NCORES = 8
HPC = (B * H) // NCORES  # heads per core = 4
NT = N // 128            # 16 (q and k 128-blocks)
MT = M // 128            # 2
NQG = 2                  # q-group count (software pipeline A||B)


def _emit(tc, ctx, io):
    nc = tc.nc
    P = 128
    GQ = NT // NQG               # q-blocks per group
    SCH = min(1024, N)           # score chunk (elements) per activation
    NCH = N // SCH               # activations per q-block

    sh = ctx.enter_context(tc.tile_pool(name="shared", bufs=1))
    exp_pool = ctx.enter_context(tc.tile_pool(name="exp", bufs=2 * GQ))
    kt_pool = ctx.enter_context(tc.tile_pool(name="ktr", bufs=2))
    qt_pool = ctx.enter_context(tc.tile_pool(name="qtr", bufs=2))
    vm_pool = ctx.enter_context(tc.tile_pool(name="vmask", bufs=2))
    cq_pool = ctx.enter_context(tc.tile_pool(name="cq", bufs=2))
    a1_pool = ctx.enter_context(tc.tile_pool(name="a1t", bufs=2))
    gt_pool = ctx.enter_context(tc.tile_pool(name="gt", bufs=2))
    vd_pool = ctx.enter_context(tc.tile_pool(name="vd", bufs=2))
    y_pool = ctx.enter_context(tc.tile_pool(name="y", bufs=2))
    ost_pool = ctx.enter_context(tc.tile_pool(name="ost", bufs=2))
    msk_pool = ctx.enter_context(tc.tile_pool(name="msk", bufs=2))
    st_pool = ctx.enter_context(tc.tile_pool(name="stats", bufs=6))

    ps_s = ctx.enter_context(tc.tile_pool(name="ps_s", bufs=2, space="PSUM"))
    ps_a1 = ctx.enter_context(tc.tile_pool(name="ps_a1", bufs=2, space="PSUM"))
    ps_gt = ctx.enter_context(tc.tile_pool(name="ps_gt", bufs=1, space="PSUM"))
    ps_m = ctx.enter_context(tc.tile_pool(name="ps_m", bufs=1, space="PSUM"))

    # --- shared, once per core ------------------------------------------
    maskB = sh.tile([64, N], F32)       # mask row broadcast over d-partitions
    nc.sync.dma_start(maskB[:], io["maskB"])

    qdtr = sh.tile([P, NT, M], F32R)    # Qd^T (Vd lhsT + CqT source)
    nc.sync.dma_start(qdtr[:], io["QdTr"].rearrange("(t p) m -> p t m", p=P))
    qdt16 = sh.tile([P, NT, M], BF16)   # Qd^T (GT lhsT)
    nc.sync.dma_start(qdt16[:], io["QdT16"].rearrange("(t p) m -> p t m", p=P))
    qdnr = sh.tile([P, MT, N], F32R)    # Qd [m,q] (out lhsT)
    nc.sync.dma_start(qdnr[:], io["QdNr"].rearrange("(c p) q -> p c q", p=P))

    state = [None] * HPC

    def prep_dma(h):
        st = state[h] = {}
        st["mk"] = msk_pool.tile([P, NT], F32, name="mk", tag="mk")
        nc.sync.dma_start(st["mk"][:], io["maskT"][h])
        st["qt"] = qt_pool.tile([64, N], F32R, name="qt", tag="qt")
        nc.sync.dma_start(st["qt"][:], io["QT"][h])
        st["vm"] = vm_pool.tile([P, NT, D], F32R, name="vm", tag="vm")
        nc.sync.dma_start(st["vm"][:], io["V"][h].rearrange("(t p) d -> p t d", p=P))
        st["kts"] = kt_pool.tile([64, N], F32R, name="kts", tag="kts")
        nc.sync.dma_start(st["kts"][:], io["KT"][h])

    def prep_compute(h):
        st = state[h]
        vm, mk, kts = st["vm"], st["mk"], st["kts"]
        for t in range(NT):
            nc.vector.tensor_scalar_mul(vm[:, t, :], vm[:, t, :], mk[:, t : t + 1])
        nc.vector.tensor_mul(kts[:], kts[:], maskB[:])
        st["cq"] = cq_pool.tile([P, NT, M], BF16, name="cq", tag="cq")
        st["a1"] = a1_pool.tile([P, NT, NQG, M], BF16, name="a1", tag="a1")
        st["exps"] = {}
        # Vd = Qd @ (V*m) -> [M, D]
        vd = st["vd"] = vd_pool.tile([P, MT, D], F32R, name="vd", tag="vd")
        for mh in range(MT):
            vps = ps_m.tile([P, D], F32, name="misc", tag="misc")
            for t in range(NT):
                nc.tensor.matmul(
                    vps[:],
                    lhsT=qdtr[:, t, mh * P : (mh + 1) * P],
                    rhs=vm[:, t, :],
                    start=(t == 0),
                    stop=(t == NT - 1),
                )
            nc.vector.tensor_copy(vd[:, mh, :], vps[:])

    def a_qblk(h, q):
        st = state[h]
        ex = exp_pool.tile([P, N], BF16, name="exp", tag="exp")
        sums = st_pool.tile([P, NCH], F32, name="sums", tag="sums")
        for c in range(NCH):
            sps = ps_s.tile([P, SCH], F32, name="s", tag="s")
            for j in range(SCH // 512):
                nc.tensor.matmul(
                    sps[:, j * 512 : (j + 1) * 512],
                    lhsT=st["qt"][:, q * P : (q + 1) * P],
                    rhs=st["kts"][:, c * SCH + j * 512 : c * SCH + (j + 1) * 512],
                    start=True,
                    stop=True,
                )
            nc.scalar.activation(
                ex[:, c * SCH : (c + 1) * SCH],
                sps[:],
                AF.Exp,
                scale=0.125,
                accum_out=sums[:, c : c + 1],
            )
        den = st_pool.tile([P, 1], F32, name="den", tag="den")
        if NCH > 1:
            nc.vector.tensor_reduce(den[:], sums[:], axis=AX.X, op=ALU.add)
        else:
            den = sums
        rec = st_pool.tile([P, 1], F32, name="rec", tag="rec")
        nc.vector.reciprocal(rec[:], den[:])
        nc.vector.tensor_scalar_mul(st["cq"][:, q, :], qdtr[:, q, :], rec[:])
        st["exps"][q] = ex

    def b_kc(h, g, kc):
        st = state[h]
        aps_ = ps_a1.tile([P, M], F32, name="a1", tag="a1")
        for qi in range(GQ):
            q = g * GQ + qi
            nc.tensor.matmul(
                aps_[:],
                lhsT=st["exps"][q][:, kc * P : (kc + 1) * P],
                rhs=st["cq"][:, q, :],
                start=(qi == 0),
                stop=(qi == GQ - 1),
            )
        nc.vector.tensor_copy(st["a1"][:, kc, g, :], aps_[:])

    def tail(h):
        st = state[h]
        gt = gt_pool.tile([P, MT, M], F32R, name="gt", tag="gt")
        gps = ps_gt.tile([P, MT * M], F32, name="g", tag="g")
        for nh in range(MT):
            for kc in range(NT):
                for g in range(NQG):
                    nc.tensor.matmul(
                        gps[:, nh * M : (nh + 1) * M],
                        lhsT=qdt16[:, kc, nh * P : (nh + 1) * P],
                        rhs=st["a1"][:, kc, g, :],
                        start=(kc == 0 and g == 0),
                        stop=(kc == NT - 1 and g == NQG - 1),
                    )
            nc.vector.tensor_copy(gt[:, nh, :], gps[:, nh * M : (nh + 1) * M])

        yt = y_pool.tile([P, MT, D], F32R, name="yt", tag="yt")
        for mh in range(MT):
            yps = ps_m.tile([P, D], F32, name="misc", tag="misc")
            for nh in range(MT):
                nc.tensor.matmul(
                    yps[:],
                    lhsT=gt[:, nh, mh * P : (mh + 1) * P],
                    rhs=st["vd"][:, nh, :],
                    start=(nh == 0),
                    stop=(nh == MT - 1),
                )
            nc.vector.tensor_copy(yt[:, mh, :], yps[:])

        o_r = io["out"][h].rearrange("(t p) d -> t p d", p=P)
        for q in range(NT):
            ops_ = ps_m.tile([P, D], F32, name="misc", tag="misc")
            for mh in range(MT):
                nc.tensor.matmul(
                    ops_[:],
                    lhsT=qdnr[:, mh, q * P : (q + 1) * P],
                    rhs=yt[:, mh, :],
                    start=(mh == 0),
                    stop=(mh == MT - 1),
                )
            ost = ost_pool.tile([P, D], F32, name="ost", tag="ost")
            nc.vector.tensor_copy(ost[:], ops_[:])
            nc.sync.dma_start(o_r[q], ost[:])
        state[h] = None

    # --- software-pipelined emission over group slots -------------------
    slots = [(h, g) for h in range(HPC) for g in range(NQG)]
    IL = NT // GQ  # B-steps per interleaved A-step (2)
    prep_dma(0)
    prep_compute(0)
    for q in range(GQ):
        a_qblk(0, q)
    for i, (h, g) in enumerate(slots):
        nxt = slots[i + 1] if i + 1 < len(slots) else None
        if g == 0 and h + 1 < HPC:
            prep_dma(h + 1)
        if g == NQG - 1 and h + 1 < HPC:
            prep_compute(h + 1)
        for kc in range(NT):
            b_kc(h, g, kc)
            if nxt is not None and kc % IL == IL - 1:
                nh_, ng_ = nxt
                a_qblk(nh_, ng_ * GQ + kc // IL)
        if g == NQG - 1:
            tail(h)


def build_nc():
    from contextlib import ExitStack

    nc = bacc.Bacc("TRN2", target_bir_lowering=False, debug=False)
    io = {
        "QT": nc.dram_tensor("QT", [HPC, 64, N], F32R, kind="ExternalInput").ap(),
        "KT": nc.dram_tensor("KT", [HPC, 64, N], F32R, kind="ExternalInput").ap(),
        "V": nc.dram_tensor("V", [HPC, N, D], F32R, kind="ExternalInput").ap(),
        "maskT": nc.dram_tensor("maskT", [HPC, 128, NT], F32, kind="ExternalInput").ap(),
        "maskB": nc.dram_tensor("maskB", [64, N], F32, kind="ExternalInput").ap(),
        "QdTr": nc.dram_tensor("QdTr", [N, M], F32R, kind="ExternalInput").ap(),
        "QdT16": nc.dram_tensor("QdT16", [N, M], BF16, kind="ExternalInput").ap(),
        "QdNr": nc.dram_tensor("QdNr", [M, N], F32R, kind="ExternalInput").ap(),
        "out": nc.dram_tensor("out", [HPC, N, D], F32, kind="ExternalOutput").ap(),
    }
    with tile.TileContext(nc) as tc:
        with ExitStack() as ctx:
            _emit(tc, ctx, io)
    nc.compile()
    return nc


_NC = None


def _get_nc():
    global _NC
    if _NC is None:
        _NC = build_nc()
    return _NC


def make_in_maps(Q, K, V, mask, Q_dct):
    Q = np.asarray(Q, dtype=np.float32).reshape(B * H, N, D)
    K = np.asarray(K, dtype=np.float32).reshape(B * H, N, D)
    V = np.asarray(V, dtype=np.float32).reshape(B * H, N, D)
    mask = np.asarray(mask, dtype=np.float32)
    Q_dct = np.asarray(Q_dct, dtype=np.float32)

    QT = np.ascontiguousarray(Q.transpose(0, 2, 1))
    KT = np.ascontiguousarray(K.transpose(0, 2, 1))
    QdT = np.ascontiguousarray(Q_dct.T)
    QdT16 = QdT.astype(NPBF16)
    QdN = np.ascontiguousarray(Q_dct)
    # maskT[b, p, t] = mask[b, t*128 + p]
    maskT = np.ascontiguousarray(mask.reshape(B, NT, 128).transpose(0, 2, 1))

    in_maps = []
    for c in range(NCORES):
        sl = slice(HPC * c, HPC * (c + 1))
        heads = range(HPC * c, HPC * (c + 1))
        in_maps.append(
            {
                "QT": np.ascontiguousarray(QT[sl]),
                "KT": np.ascontiguousarray(KT[sl]),
                "V": np.ascontiguousarray(V[sl]),
                "maskT": np.ascontiguousarray(
                    np.stack([maskT[hp // H] for hp in heads])
                ),
                "maskB": np.ascontiguousarray(
                    np.broadcast_to(mask[(HPC * c) // H][None, :], (64, N))
                ),
                "QdTr": QdT,
                "QdT16": QdT16,
                "QdNr": QdN,
            }
        )
    return in_maps


def run_on_device(in_maps, **kwargs):
    nc = _get_nc()
    return bass_utils.run_bass_kernel_spmd(
        nc, in_maps, core_ids=list(range(NCORES)), **kwargs
    )


def kernel(Q, K, V, mask, Q_dct):
    in_maps = make_in_maps(Q, K, V, mask, Q_dct)
    res = run_on_device(in_maps)
    out = np.empty((B * H, N, D), dtype=np.float32)
    for c in range(NCORES):
        out[HPC * c : HPC * (c + 1)] = res.results[c]["out"]
    return out.reshape(B, H, N, D)



# revision 8
# speedup vs baseline: 1.3155x; 1.3155x over previous
"""DCT-compressed attention (nn_DCTAttentionIdeal) on 8 Trainium2 NeuronCores.

Math (per head, reference ordering):
    S    = (Q*s) @ (K*mask*s)^T with s = D**-0.25             [N,N]
    atn  = softmax(S, axis=-1)
    Vd   = Qd @ (V*mask)                                      [M,D]
    out  = Qd^T @ ((Qd @ atn @ Qd^T) @ Vd)                    [N,D]

Kernel reassociation (exact in real arithmetic):
    out = Qd^T @ (Qd @ (atn @ U)),   U = Qd^T @ (Qd @ (V*mask))  [N,D]
so the only O(N^2) contractions are the scores (N^2 D) and atn @ U
(N^2 D) -- the N^2 M path of the naive ordering is gone.

Implementation notes:
  - scores are computed TRANSPOSED (S^T[k,q]) so that exp(S^T) tiles act
    directly as matmul weights for the k-contraction of atn @ U, and the
    softmax denominator comes free as a ones-column appended to U.
  - exp activation folds everything: exp(0.125*mask[k]*Sraw - 2.5)
    (per-partition scale AP carries the key mask; the -2.5 shift keeps
    values in a comfortable range; it cancels in the softmax ratio).
  - exp is stored bf16 (fp8 mantissa is too coarse: exp quantization
    error passes straight through atn@U since Y is a random-sign sum).
  - S^T uses 2x PE row tiling (contraction D=64): even k-blocks on array
    rows 0-63, odd k-blocks on rows 64-127, running concurrently.
  - PSUM: 6 banks = 3-deep rotation of [128,1024] score regions (keeps
    ScalarE 100% fed), 1 bank atn@U accumulator, 1 bank misc tail.

Sharding: batch*heads (2*16=32) split 4-per-core across 8 cores; Q_dct
replicated; no cross-core communication.  Host does layout only
(transposes / duplication for row tiling); masking, softmax and all
DCT algebra run on-device.
"""

import numpy as np
import ml_dtypes

import concourse.tile as tile
from concourse import bacc, mybir
from concourse import bass_utils

F32 = mybir.dt.float32
F32R = mybir.dt.float32r
BF16 = mybir.dt.bfloat16
FP8 = mybir.dt.float8e4
NPBF16 = ml_dtypes.bfloat16
AF = mybir.ActivationFunctionType

B, H, N, D, M = 2, 16, 2048, 64, 256
NCORES = 8
HPC = (B * H) // NCORES  # heads per core = 4
P = 128
NT = N // P              # 16 k/q blocks
MT = M // P              # 2
EXP_BIAS = -2.5          # exp(s - 2.5): keeps fp8e4 exp values in range


def _emit(tc, ctx, io):
    nc = tc.nc

    sh = ctx.enter_context(tc.tile_pool(name="shared", bufs=1))
    in2 = ctx.enter_context(tc.tile_pool(name="inputs", bufs=3))
    exp_pool = ctx.enter_context(tc.tile_pool(name="exp", bufs=2))
    ua_pool = ctx.enter_context(tc.tile_pool(name="ua", bufs=2))
    vd_pool = ctx.enter_context(tc.tile_pool(name="vd", bufs=2))
    au_pool = ctx.enter_context(tc.tile_pool(name="atnun", bufs=2))
    z1_pool = ctx.enter_context(tc.tile_pool(name="z1", bufs=2))
    ost_pool = ctx.enter_context(tc.tile_pool(name="ost", bufs=1))
    st_pool = ctx.enter_context(tc.tile_pool(name="stats", bufs=4))

    ps_st = ctx.enter_context(tc.tile_pool(name="ps_st", bufs=3, space="PSUM"))
    ps_a = ctx.enter_context(tc.tile_pool(name="ps_a", bufs=1, space="PSUM"))
    ps_m = ctx.enter_context(tc.tile_pool(name="ps_m", bufs=1, space="PSUM"))

    # --- shared, once per core ------------------------------------------
    qdtr = sh.tile([P, NT, M], BF16)    # Qd^T[n, m] -> [p, t, m]
    nc.sync.dma_start(qdtr[:], io["QdTr"].rearrange("(t p) m -> p t m", p=P))
    qdnr = sh.tile([P, MT, N], BF16)    # Qd[m, q]   -> [p, c, q]
    nc.sync.dma_start(qdnr[:], io["QdNr"].rearrange("(c p) q -> p c q", p=P))
    ebias = sh.tile([P, 1], F32)        # exp bias column (const)
    nc.vector.memset(ebias[:], EXP_BIAS)

    state = [None] * HPC

    def prep_dma(h):
        st = state[h] = {}
        st["qt2"] = in2.tile([P, N], BF16, name="qt2", tag="qt2")
        nc.sync.dma_start(st["qt2"][:], io["QT2"][h])
        st["kt2"] = in2.tile([P, NT // 2, P], BF16, name="kt2", tag="kt2")
        nc.sync.dma_start(st["kt2"][:], io["KT2"][h])
        st["v"] = in2.tile([P, NT, D], BF16, name="v", tag="v")
        nc.sync.dma_start(st["v"][:], io["V"][h].rearrange("(t p) d -> p t d", p=P))
        st["mv"] = in2.tile([P, NT], F32, name="mv", tag="mv")
        nc.sync.dma_start(st["mv"][:], io["maskV"][h])
        st["ms"] = in2.tile([P, NT], F32, name="ms", tag="ms")
        nc.sync.dma_start(st["ms"][:], io["maskS"][h])

    def prep_compute(h):
        """V masking + Vd = Qd@(V*m) + U = Qd^T@Vd (+ ones column)."""
        st = state[h]
        v, mv = st["v"], st["mv"]
        for t in range(NT):
            nc.vector.tensor_scalar_mul(v[:, t, :], v[:, t, :], mv[:, t : t + 1])
        # Vd [M, D] -> [128, 2, 64]
        vd = st["vd"] = vd_pool.tile([P, MT, D], BF16, name="vd", tag="vd")
        psv = ps_m.tile([P, 512], F32, name="psv", tag="misc")
        for mh in range(MT):
            for t in range(NT):
                nc.tensor.matmul(
                    psv[:, mh * D : (mh + 1) * D],
                    lhsT=qdtr[:, t, mh * P : (mh + 1) * P],
                    rhs=v[:, t, :],
                    start=(t == 0),
                    stop=(t == NT - 1),
                )
        nc.vector.tensor_copy(vd[:], psv[:, 0 : MT * D].rearrange("p (c d) -> p c d", d=D))
        # U [N, D] (+ ones col 64) -> ua [128, 16, 65]
        ua = st["ua"] = ua_pool.tile([P, NT, D + 1], BF16, name="ua", tag="ua")
        for half in range(2):
            psu = ps_m.tile([P, 512], F32, name="psu", tag="misc")
            for k8 in range(8):
                kc = half * 8 + k8
                for mh in range(MT):
                    nc.tensor.matmul(
                        psu[:, k8 * D : (k8 + 1) * D],
                        lhsT=qdnr[:, mh, kc * P : (kc + 1) * P],
                        rhs=vd[:, mh, :],
                        start=(mh == 0),
                        stop=(mh == MT - 1),
                    )
            nc.vector.tensor_copy(
                ua[:, half * 8 : (half + 1) * 8, 0:D],
                psu[:].rearrange("p (c d) -> p c d", d=D),
            )
        nc.vector.memset(ua[:, :, D : D + 1], 1.0)

    def s_pair(h, j):
        """Scores^T + exp for k-blocks (2j, 2j+1); row-tiled across the PE."""
        st = state[h]
        ex, kt2, qt2, ms = st["ex"], st["kt2"], st["qt2"], st["ms"]
        for side in range(2):
            kc = 2 * j + side
            pr = slice(64 * side, 64 * (side + 1))
            for half in range(2):
                ps = ps_st.tile([P, 1024], F32, name="st", tag="st")
                for c in range(2):
                    q0 = half * 1024 + c * 512
                    nc.tensor.matmul(
                        ps[:, c * 512 : (c + 1) * 512],
                        lhsT=kt2[pr, j, :],
                        rhs=qt2[pr, q0 : q0 + 512],
                        start=True,
                        stop=True,
                    )
                nc.scalar.activation(
                    ex[:, kc, half * 1024 : (half + 1) * 1024],
                    ps[:],
                    AF.Exp,
                    bias=ebias[:],
                    scale=ms[:, kc : kc + 1],
                )

    def atnu_q(h, q):
        """One q-block of Y = atn @ [U|1]: accumulate over all 16 k-blocks,
        then normalize by the ones-column denominator."""
        st = state[h]
        ex, ua = st["ex"], st["ua"]
        psa = ps_a.tile([P, D + 1], F32, name="psa", tag="psa")
        for kc in range(NT):
            nc.tensor.matmul(
                psa[:],
                lhsT=ex[:, kc, q * P : (q + 1) * P],
                rhs=ua[:, kc, :],
                start=(kc == 0),
                stop=(kc == NT - 1),
            )
        rec = st_pool.tile([P, 1], F32, name="rec", tag="rec")
        nc.vector.reciprocal(rec[:], psa[:, D : D + 1])
        nc.vector.tensor_scalar_mul(st["au"][:, q, :], psa[:, 0:D], rec[:])

    def tail(h):
        """Z1 = Qd @ Y; out = Qd^T @ Z1; DMA out."""
        st = state[h]
        au = st["au"]
        z1 = z1_pool.tile([P, MT, D], BF16, name="z1", tag="z1")
        psz = ps_m.tile([P, 512], F32, name="psz", tag="misc")
        for mh in range(MT):
            for q in range(NT):
                nc.tensor.matmul(
                    psz[:, mh * D : (mh + 1) * D],
                    lhsT=qdtr[:, q, mh * P : (mh + 1) * P],
                    rhs=au[:, q, :],
                    start=(q == 0),
                    stop=(q == NT - 1),
                )
        nc.vector.tensor_copy(z1[:], psz[:, 0 : MT * D].rearrange("p (c d) -> p c d", d=D))
        ost = ost_pool.tile([P, NT, D], F32, name="ost", tag="ost")
        for half in range(2):
            pso = ps_m.tile([P, 512], F32, name="pso", tag="misc")
            for q8 in range(8):
                q = half * 8 + q8
                for mh in range(MT):
                    nc.tensor.matmul(
                        pso[:, q8 * D : (q8 + 1) * D],
                        lhsT=qdnr[:, mh, q * P : (q + 1) * P],
                        rhs=z1[:, mh, :],
                        start=(mh == 0),
                        stop=(mh == MT - 1),
                    )
            nc.vector.tensor_copy(
                ost[:, half * 8 : (half + 1) * 8, :],
                pso[:].rearrange("p (c d) -> p c d", d=D),
            )
        nc.sync.dma_start(io["out"][h].rearrange("(t p) d -> p t d", p=P), ost[:])
        state[h] = None

    # --- pipelined emission --------------------------------------------
    prep_dma(0)
    if HPC > 1:
        prep_dma(1)
    for h in range(HPC):
        st = state[h]
        st["ex"] = exp_pool.tile([P, NT, N], BF16, name="ex", tag="ex")
        st["au"] = au_pool.tile([P, NT, D], BF16, name="au", tag="au")
        prep_compute(h)
        for j in range(NT // 2):
            if j == 4 and h + 2 < HPC:
                prep_dma(h + 2)
            s_pair(h, j)
            if h > 0:
                atnu_q(h - 1, 2 * j)
                atnu_q(h - 1, 2 * j + 1)
        if h > 0:
            tail(h - 1)
    for q in range(NT):
        atnu_q(HPC - 1, q)
    tail(HPC - 1)


def build_nc():
    from contextlib import ExitStack

    nc = bacc.Bacc("TRN2", target_bir_lowering=False, debug=False)
    io = {
        "QT2": nc.dram_tensor("QT2", [HPC, P, N], BF16, kind="ExternalInput").ap(),
        "KT2": nc.dram_tensor("KT2", [HPC, P, NT // 2, P], BF16, kind="ExternalInput").ap(),
        "V": nc.dram_tensor("V", [HPC, N, D], BF16, kind="ExternalInput").ap(),
        "maskV": nc.dram_tensor("maskV", [HPC, P, NT], F32, kind="ExternalInput").ap(),
        "maskS": nc.dram_tensor("maskS", [HPC, P, NT], F32, kind="ExternalInput").ap(),
        "QdTr": nc.dram_tensor("QdTr", [N, M], BF16, kind="ExternalInput").ap(),
        "QdNr": nc.dram_tensor("QdNr", [M, N], BF16, kind="ExternalInput").ap(),
        "out": nc.dram_tensor("out", [HPC, N, D], F32, kind="ExternalOutput").ap(),
    }
    with tile.TileContext(nc) as tc:
        with ExitStack() as ctx:
            _emit(tc, ctx, io)
    nc.compile()
    return nc


_NC = None


def _get_nc():
    global _NC
    if _NC is None:
        _NC = build_nc()
    return _NC


def make_in_maps(Q, K, V, mask, Q_dct):
    Q = np.asarray(Q, dtype=np.float32).reshape(B * H, N, D)
    K = np.asarray(K, dtype=np.float32).reshape(B * H, N, D)
    V = np.asarray(V, dtype=np.float32).reshape(B * H, N, D)
    mask = np.asarray(mask, dtype=np.float32)
    Q_dct = np.asarray(Q_dct, dtype=np.float32)

    QT = np.ascontiguousarray(Q.transpose(0, 2, 1))          # [BH, 64, N]
    KT = np.ascontiguousarray(K.transpose(0, 2, 1))          # [BH, 64, N]
    # duplicate Q^T across both partition halves (PE row tiling)
    QT2 = np.concatenate([QT, QT], axis=1)                   # [BH, 128, N]
    # interleave K^T k-blocks: even blocks on partitions 0-63, odd on 64-127
    KTb = KT.reshape(B * H, D, NT, P)                        # [BH, 64, 16, 128]
    KT2 = np.concatenate([KTb[:, :, 0::2, :], KTb[:, :, 1::2, :]], axis=1)
    # mask per (partition, block): mask[b, t*128 + p]
    maskT = np.ascontiguousarray(mask.reshape(B, NT, P).transpose(0, 2, 1))
    QdTr = np.ascontiguousarray(Q_dct.T).astype(NPBF16)
    QdNr = np.ascontiguousarray(Q_dct).astype(NPBF16)

    in_maps = []
    for c in range(NCORES):
        sl = slice(HPC * c, HPC * (c + 1))
        heads = range(HPC * c, HPC * (c + 1))
        mT = np.stack([maskT[hp // H] for hp in heads])      # [HPC, 128, 16]
        in_maps.append(
            {
                "QT2": np.ascontiguousarray(QT2[sl]).astype(NPBF16),
                "KT2": np.ascontiguousarray(KT2[sl]).astype(NPBF16),
                "V": np.ascontiguousarray(V[sl]).astype(NPBF16),
                "maskV": np.ascontiguousarray(mT),
                "maskS": np.ascontiguousarray(0.125 * mT),
                "QdTr": QdTr,
                "QdNr": QdNr,
            }
        )
    return in_maps


def run_on_device(in_maps, **kwargs):
    nc = _get_nc()
    return bass_utils.run_bass_kernel_spmd(
        nc, in_maps, core_ids=list(range(NCORES)), **kwargs
    )


def kernel(Q, K, V, mask, Q_dct):
    in_maps = make_in_maps(Q, K, V, mask, Q_dct)
    res = run_on_device(in_maps)
    out = np.empty((B * H, N, D), dtype=np.float32)
    for c in range(NCORES):
        out[HPC * c : HPC * (c + 1)] = res.results[c]["out"]
    return out.reshape(B, H, N, D)


# revision 9
# speedup vs baseline: 1.3673x; 1.0394x over previous
"""DCT-compressed attention (nn_DCTAttentionIdeal) on 8 Trainium2 NeuronCores.

Math (per head, reference ordering):
    S    = (Q*s) @ (K*mask*s)^T with s = D**-0.25             [N,N]
    atn  = softmax(S, axis=-1)
    Vd   = Qd @ (V*mask)                                      [M,D]
    out  = Qd^T @ ((Qd @ atn @ Qd^T) @ Vd)                    [N,D]

Kernel reassociation (exact in real arithmetic):
    out = Qd^T @ (Qd @ (atn @ U)),   U = Qd^T @ (Qd @ (V*mask))  [N,D]
so the only O(N^2) contractions are the scores (N^2 D) and atn @ U
(N^2 D) -- the N^2 M path of the naive ordering is gone.

Implementation notes:
  - scores are computed TRANSPOSED (S^T[k,q]) so that exp(S^T) tiles act
    directly as matmul weights for the k-contraction of atn @ U, and the
    softmax denominator comes free as a ones-column appended to U.
  - exp is stored bf16 (fp8 mantissa is too coarse: exp quantization
    error passes straight through atn@U since Y is a random-sign sum).
  - S^T uses 2x PE row tiling (contraction D=64): even k-blocks on array
    rows 0-63, odd k-blocks on rows 64-127.
  - The whole kernel is paced by ScalarE's exp stream (128 activations of
    [128,1024]).  Emission is act-paced: each score unit emits 2 matmuls
    + 1 activation, then pops one "filler atom" (a ~16-matmul slice of
    the atn@U / DCT-tail / U-prep work for neighboring heads), so the
    strict-FIFO TensorE queue never blocks on the act-region rotation
    with runnable work stuck behind it.
  - PSUM: 6 banks = 3-deep rotation of [128,1024] score regions, 1 bank
    atn@U accumulator, 1 bank misc (Vd/U/Z1/out tail).

Sharding: batch*heads (2*16=32) split 4-per-core across 8 cores; Q_dct
replicated; no cross-core communication.  Host does layout only
(transposes / duplication for row tiling / mask broadcast layouts);
masking, softmax and all DCT algebra run on-device.
"""

from collections import deque

import numpy as np
import ml_dtypes

import concourse.tile as tile
from concourse import bacc, mybir
from concourse import bass_utils

F32 = mybir.dt.float32
F32R = mybir.dt.float32r
BF16 = mybir.dt.bfloat16
NPBF16 = ml_dtypes.bfloat16
AF = mybir.ActivationFunctionType

B, H, N, D, M = 2, 16, 2048, 64, 256
NCORES = 8
HPC = (B * H) // NCORES  # heads per core = 4
P = 128
NT = N // P              # 16 k/q blocks
MT = M // P              # 2
EXP_BIAS = -2.5          # exp(s - 2.5): comfortable value range; cancels in ratio


def _emit(tc, ctx, io):
    nc = tc.nc

    sh = ctx.enter_context(tc.tile_pool(name="shared", bufs=1))
    in2 = ctx.enter_context(tc.tile_pool(name="inputs", bufs=3))
    exp_pool = ctx.enter_context(tc.tile_pool(name="exp", bufs=2))
    ua_pool = ctx.enter_context(tc.tile_pool(name="ua", bufs=2))
    vd_pool = ctx.enter_context(tc.tile_pool(name="vd", bufs=2))
    au_pool = ctx.enter_context(tc.tile_pool(name="atnun", bufs=2))
    z1_pool = ctx.enter_context(tc.tile_pool(name="z1", bufs=2))
    ost_pool = ctx.enter_context(tc.tile_pool(name="ost", bufs=1))
    st_pool = ctx.enter_context(tc.tile_pool(name="stats", bufs=4))

    ps_st = ctx.enter_context(tc.tile_pool(name="ps_st", bufs=3, space="PSUM"))
    ps_a = ctx.enter_context(tc.tile_pool(name="ps_a", bufs=1, space="PSUM"))
    ps_m = ctx.enter_context(tc.tile_pool(name="ps_m", bufs=1, space="PSUM"))

    # --- shared, once per core ------------------------------------------
    qdtr = sh.tile([P, NT, M], BF16)    # Qd^T[n, m] -> [p, t, m]
    nc.sync.dma_start(qdtr[:], io["QdTr"].rearrange("(t p) m -> p t m", p=P))
    qdnr = sh.tile([P, MT, N], BF16)    # Qd[m, q]   -> [p, c, q]
    nc.sync.dma_start(qdnr[:], io["QdNr"].rearrange("(c p) q -> p c q", p=P))
    mkb = sh.tile([P, NT // 2, P], BF16)  # key mask in KT2 layout
    nc.sync.dma_start(mkb[:], io["maskKB"])
    mvb = sh.tile([P, NT, D], BF16)       # value mask in V layout
    nc.sync.dma_start(mvb[:], io["maskVB"])
    ebias = sh.tile([P, 1], F32)          # exp bias column (const)
    nc.vector.memset(ebias[:], EXP_BIAS)

    state = [None] * HPC

    def prep_dma(h):
        st = state[h] = {}
        st["qt2"] = in2.tile([P, N], BF16, name="qt2", tag="qt2")
        nc.sync.dma_start(st["qt2"][:], io["QT2"][h])
        st["kt2"] = in2.tile([P, NT // 2, P], BF16, name="kt2", tag="kt2")
        nc.sync.dma_start(st["kt2"][:], io["KT2"][h])
        st["v"] = in2.tile([P, NT, D], BF16, name="v", tag="v")
        nc.sync.dma_start(st["v"][:], io["V"][h].rearrange("(t p) d -> p t d", p=P))

    def mask_inputs(h):
        st = state[h]
        nc.vector.tensor_mul(st["kt2"][:], st["kt2"][:], mkb[:])
        nc.vector.tensor_mul(st["v"][:], st["v"][:], mvb[:])

    # ---- filler atoms (TensorE work hidden inside the act-paced spine) --
    def vd_atom(h):
        """Vd = Qd @ (V*m) -> [128, 2, 64]."""
        st = state[h]
        vd = st["vd"] = vd_pool.tile([P, MT, D], BF16, name="vd", tag="vd")
        psv = ps_m.tile([P, 512], F32, name="psv", tag="misc")
        for mh in range(MT):
            for t in range(NT):
                nc.tensor.matmul(
                    psv[:, mh * D : (mh + 1) * D],
                    lhsT=qdtr[:, t, mh * P : (mh + 1) * P],
                    rhs=st["v"][:, t, :],
                    start=(t == 0),
                    stop=(t == NT - 1),
                )
        nc.vector.tensor_copy(vd[:], psv[:, 0 : MT * D].rearrange("p (c d) -> p c d", d=D))

    def u_atom(h, half):
        """U = Qd^T @ Vd (one half of 16 k-blocks) + ones column."""
        st = state[h]
        if half == 0:
            st["ua"] = ua_pool.tile([P, NT, D + 1], BF16, name="ua", tag="ua")
        ua = st["ua"]
        psu = ps_m.tile([P, 512], F32, name="psu", tag="misc")
        for k8 in range(8):
            kc = half * 8 + k8
            for mh in range(MT):
                nc.tensor.matmul(
                    psu[:, k8 * D : (k8 + 1) * D],
                    lhsT=qdnr[:, mh, kc * P : (kc + 1) * P],
                    rhs=st["vd"][:, mh, :],
                    start=(mh == 0),
                    stop=(mh == MT - 1),
                )
        nc.vector.tensor_copy(
            ua[:, half * 8 : (half + 1) * 8, 0:D],
            psu[:].rearrange("p (c d) -> p c d", d=D),
        )
        nc.vector.memset(ua[:, half * 8 : (half + 1) * 8, D : D + 1], 1.0)

    def atnu_atom(h, q):
        """One q-block of Y = atn @ [U|1]: 16 matmuls + normalize."""
        st = state[h]
        if q == 0:
            st["au"] = au_pool.tile([P, NT, D], BF16, name="au", tag="au")
        psa = ps_a.tile([P, D + 1], F32, name="psa", tag="psa")
        for kc in range(NT):
            nc.tensor.matmul(
                psa[:],
                lhsT=st["ex"][:, kc, q * P : (q + 1) * P],
                rhs=st["ua"][:, kc, :],
                start=(kc == 0),
                stop=(kc == NT - 1),
            )
        rec = st_pool.tile([P, 1], F32, name="rec", tag="rec")
        nc.vector.reciprocal(rec[:], psa[:, D : D + 1])
        nc.vector.tensor_scalar_mul(st["au"][:, q, :], psa[:, 0:D], rec[:])

    def z1_atom(h):
        """Z1 = Qd @ Y -> [128, 2, 64]."""
        st = state[h]
        z1 = st["z1"] = z1_pool.tile([P, MT, D], BF16, name="z1", tag="z1")
        psz = ps_m.tile([P, 512], F32, name="psz", tag="misc")
        for mh in range(MT):
            for q in range(NT):
                nc.tensor.matmul(
                    psz[:, mh * D : (mh + 1) * D],
                    lhsT=qdtr[:, q, mh * P : (mh + 1) * P],
                    rhs=st["au"][:, q, :],
                    start=(q == 0),
                    stop=(q == NT - 1),
                )
        nc.vector.tensor_copy(z1[:], psz[:, 0 : MT * D].rearrange("p (c d) -> p c d", d=D))

    def out_atom(h, half):
        """out = Qd^T @ Z1 for 8 q-blocks; DMA at the end."""
        st = state[h]
        if half == 0:
            st["ost"] = ost_pool.tile([P, NT, D], F32, name="ost", tag="ost")
        ost = st["ost"]
        pso = ps_m.tile([P, 512], F32, name="pso", tag="misc")
        for q8 in range(8):
            q = half * 8 + q8
            for mh in range(MT):
                nc.tensor.matmul(
                    pso[:, q8 * D : (q8 + 1) * D],
                    lhsT=qdnr[:, mh, q * P : (q + 1) * P],
                    rhs=st["z1"][:, mh, :],
                    start=(mh == 0),
                    stop=(mh == MT - 1),
                )
        nc.vector.tensor_copy(
            ost[:, half * 8 : (half + 1) * 8, :],
            pso[:].rearrange("p (c d) -> p c d", d=D),
        )
        if half == 1:
            nc.sync.dma_start(io["out"][h].rearrange("(t p) d -> p t d", p=P), ost[:])
            state[h] = None

    def phase_atoms(h):
        atoms = []
        if h < HPC:
            atoms.append(lambda h=h: vd_atom(h))
            for half in range(2):
                atoms.append(lambda h=h, half=half: u_atom(h, half))
        if 0 <= h - 1 < HPC:
            for q in range(NT):
                atoms.append(lambda h=h, q=q: atnu_atom(h - 1, q))
        if h - 2 >= 0:
            atoms.append(lambda h=h: z1_atom(h - 2))
            for half in range(2):
                atoms.append(lambda h=h, half=half: out_atom(h - 2, half))
        return atoms

    def s_unit(h, j, side, half):
        """One score unit: k-block 2j+side, q columns [half*1024, ...)."""
        st = state[h]
        pr = slice(64 * side, 64 * (side + 1))
        kc = 2 * j + side
        ps = ps_st.tile([P, 1024], F32, name="st", tag="st")
        for c in range(2):
            q0 = half * 1024 + c * 512
            nc.tensor.matmul(
                ps[:, c * 512 : (c + 1) * 512],
                lhsT=st["kt2"][pr, j, :],
                rhs=st["qt2"][pr, q0 : q0 + 512],
                start=True,
                stop=True,
            )
        nc.scalar.activation(
            st["ex"][:, kc, half * 1024 : (half + 1) * 1024],
            ps[:],
            AF.Exp,
            bias=ebias[:],
            scale=0.125,
        )

    # --- act-paced pipelined emission -----------------------------------
    prep_dma(0)
    if HPC > 1:
        prep_dma(1)
    mask_inputs(0)
    atoms = deque()
    for h in range(HPC):
        st = state[h]
        st["ex"] = exp_pool.tile([P, NT, N], BF16, name="ex", tag="ex")
        if h + 1 < HPC:
            mask_inputs(h + 1)
        atoms.extend(phase_atoms(h))
        for u in range(2 * NT):
            j, r = divmod(u, 4)
            side, half = r % 2, r // 2
            if u == NT and h + 2 < HPC:
                prep_dma(h + 2)
            s_unit(h, j, side, half)
            if atoms:
                atoms.popleft()()
    atoms.extend(phase_atoms(HPC))
    atoms.extend(phase_atoms(HPC + 1))
    while atoms:
        atoms.popleft()()


def build_nc():
    from contextlib import ExitStack

    nc = bacc.Bacc("TRN2", target_bir_lowering=False, debug=False)
    io = {
        "QT2": nc.dram_tensor("QT2", [HPC, P, N], BF16, kind="ExternalInput").ap(),
        "KT2": nc.dram_tensor("KT2", [HPC, P, NT // 2, P], BF16, kind="ExternalInput").ap(),
        "V": nc.dram_tensor("V", [HPC, N, D], BF16, kind="ExternalInput").ap(),
        "maskKB": nc.dram_tensor("maskKB", [P, NT // 2, P], BF16, kind="ExternalInput").ap(),
        "maskVB": nc.dram_tensor("maskVB", [P, NT, D], BF16, kind="ExternalInput").ap(),
        "QdTr": nc.dram_tensor("QdTr", [N, M], BF16, kind="ExternalInput").ap(),
        "QdNr": nc.dram_tensor("QdNr", [M, N], BF16, kind="ExternalInput").ap(),
        "out": nc.dram_tensor("out", [HPC, N, D], F32, kind="ExternalOutput").ap(),
    }
    with tile.TileContext(nc) as tc:
        with ExitStack() as ctx:
            _emit(tc, ctx, io)
    nc.compile()
    return nc


_NC = None


def _get_nc():
    global _NC
    if _NC is None:
        _NC = build_nc()
    return _NC


def make_in_maps(Q, K, V, mask, Q_dct):
    Q = np.asarray(Q, dtype=np.float32).reshape(B * H, N, D)
    K = np.asarray(K, dtype=np.float32).reshape(B * H, N, D)
    V = np.asarray(V, dtype=np.float32).reshape(B * H, N, D)
    mask = np.asarray(mask, dtype=np.float32)
    Q_dct = np.asarray(Q_dct, dtype=np.float32)

    QT = np.ascontiguousarray(Q.transpose(0, 2, 1))          # [BH, 64, N]
    KT = np.ascontiguousarray(K.transpose(0, 2, 1))          # [BH, 64, N]
    # duplicate Q^T across both partition halves (PE row tiling)
    QT2 = np.concatenate([QT, QT], axis=1)                   # [BH, 128, N]
    # interleave K^T k-blocks: even blocks on partitions 0-63, odd on 64-127
    KTb = KT.reshape(B * H, D, NT, P)                        # [BH, 64, 16, 128]
    KT2 = np.concatenate([KTb[:, :, 0::2, :], KTb[:, :, 1::2, :]], axis=1)
    QdTr = np.ascontiguousarray(Q_dct.T).astype(NPBF16)
    QdNr = np.ascontiguousarray(Q_dct).astype(NPBF16)

    in_maps = []
    for c in range(NCORES):
        sl = slice(HPC * c, HPC * (c + 1))
        heads = list(range(HPC * c, HPC * (c + 1)))
        bs = {hp // H for hp in heads}
        assert len(bs) == 1, "all heads on a core must share a batch row"
        b = bs.pop()
        # key mask in KT2 layout [128, 8, 128] (broadcast over d-partitions)
        mk = mask[b].reshape(NT, P)                          # [16 blocks, 128]
        mkb = np.empty((P, NT // 2, P), dtype=np.float32)
        mkb[0:64] = mk[0::2][None, :, :]
        mkb[64:128] = mk[1::2][None, :, :]
        # value mask in V-tile layout [128, 16, 64]
        mvb = np.broadcast_to(mk.T[:, :, None], (P, NT, D))
        in_maps.append(
            {
                "QT2": np.ascontiguousarray(QT2[sl]).astype(NPBF16),
                "KT2": np.ascontiguousarray(KT2[sl]).astype(NPBF16),
                "V": np.ascontiguousarray(V[sl]).astype(NPBF16),
                "maskKB": np.ascontiguousarray(mkb).astype(NPBF16),
                "maskVB": np.ascontiguousarray(mvb).astype(NPBF16),
                "QdTr": QdTr,
                "QdNr": QdNr,
            }
        )
    return in_maps


def run_on_device(in_maps, **kwargs):
    nc = _get_nc()
    return bass_utils.run_bass_kernel_spmd(
        nc, in_maps, core_ids=list(range(NCORES)), **kwargs
    )


def kernel(Q, K, V, mask, Q_dct):
    in_maps = make_in_maps(Q, K, V, mask, Q_dct)
    res = run_on_device(in_maps)
    out = np.empty((B * H, N, D), dtype=np.float32)
    for c in range(NCORES):
        out[HPC * c : HPC * (c + 1)] = res.results[c]["out"]
    return out.reshape(B, H, N, D)


# revision 12
# speedup vs baseline: 1.5979x; 1.1687x over previous
"""DCT-compressed attention (nn_DCTAttentionIdeal) on 8 Trainium2 NeuronCores.

Math (per head, reference ordering):
    S    = (Q*s) @ (K*mask*s)^T with s = D**-0.25             [N,N]
    atn  = softmax(S, axis=-1)
    Vd   = Qd @ (V*mask)                                      [M,D]
    out  = Qd^T @ ((Qd @ atn @ Qd^T) @ Vd)                    [N,D]

Kernel reassociation (exact in real arithmetic):
    out = Qd^T @ (Qd @ (atn @ U)),   U = Qd^T @ (Qd @ (V*mask))  [N,D]
so the only O(N^2) contractions are the scores (N^2 D) and atn @ U
(N^2 D) -- the N^2 M path of the naive ordering is gone.

Implementation notes:
  - scores are computed TRANSPOSED (S^T[k,q]) so that exp(S^T) tiles act
    directly as matmul weights for the k-contraction of atn @ U, and the
    softmax denominator comes free as a ones-column appended to U.
  - exp is stored bf16 (fp8 mantissa is too coarse: exp quantization
    error passes straight through atn@U since Y is a random-sign sum).
  - S^T uses 2x PE row tiling (contraction D=64): even k-blocks on array
    rows 0-63, odd k-blocks on rows 64-127, chunk matmuls interleaved
    across row groups so every LDWEIGHTS hides under the other group's
    stream.
  - The kernel is paced by ScalarE's exp stream (128 activations of
    [128,1024]).  Emission is act-paced: each pair-half slot emits 4
    score matmuls + 2 activations, then pops two "filler atoms" (~8-16
    matmul slices of atn@U / DCT-tail / U-prep work for neighboring
    heads), so the strict-FIFO TensorE queue never idles on the
    act-region rotation with runnable work stuck behind it.
  - atn@U accumulators alternate between two PSUM banks so the DVE
    normalize of block q never blocks the matmuls of block q+1.
  - heads are processed in pairs for the small DCT matmuls (Vd/U/Z1/out):
    two heads' 64-wide streams share one 128-wide weight load.
  - PSUM: 6 banks = 3-deep rotation of [128,1024] score regions, 2 banks
    for the alternating atn@U accumulator / misc tail.

Sharding: batch*heads (2*16=32) split 4-per-core across 8 cores; Q_dct
replicated; no cross-core communication.  Host does layout only
(transposes / duplication for row tiling / mask broadcast layouts);
masking, softmax and all DCT algebra run on-device.
"""

from collections import deque

import numpy as np
import ml_dtypes

import concourse.tile as tile
from concourse import bacc, mybir
from concourse import bass_utils

F32 = mybir.dt.float32
BF16 = mybir.dt.bfloat16
NPBF16 = ml_dtypes.bfloat16
AF = mybir.ActivationFunctionType

B, H, N, D, M = 2, 16, 2048, 64, 256
NCORES = 8
HPC = (B * H) // NCORES  # heads per core = 4
P = 128
NT = N // P              # 16 k/q blocks
MT = M // P              # 2


def _emit(tc, ctx, io):
    nc = tc.nc

    sh = ctx.enter_context(tc.tile_pool(name="shared", bufs=1))
    in2 = ctx.enter_context(tc.tile_pool(name="inputs", bufs=3))
    v2_pool = ctx.enter_context(tc.tile_pool(name="vpair", bufs=2))
    exp_pool = ctx.enter_context(tc.tile_pool(name="exp", bufs=2))
    ua_pool = ctx.enter_context(tc.tile_pool(name="ua", bufs=2))
    vd_pool = ctx.enter_context(tc.tile_pool(name="vd", bufs=2))
    au_pool = ctx.enter_context(tc.tile_pool(name="atnun", bufs=2))
    z1_pool = ctx.enter_context(tc.tile_pool(name="z1", bufs=2))
    ost_pool = ctx.enter_context(tc.tile_pool(name="ost", bufs=1))
    st_pool = ctx.enter_context(tc.tile_pool(name="stats", bufs=4))

    ps_st = ctx.enter_context(tc.tile_pool(name="ps_st", bufs=3, space="PSUM"))
    ps_a = ctx.enter_context(tc.tile_pool(name="ps_a", bufs=1, space="PSUM"))
    ps_m = ctx.enter_context(tc.tile_pool(name="ps_m", bufs=1, space="PSUM"))

    # --- shared, once per core (mask layouts first: first act needs them)
    mkb = sh.tile([P, NT // 2, P], BF16)  # key mask in KT2 layout
    nc.sync.dma_start(mkb[:], io["maskKB"])
    mvb = sh.tile([P, NT, D], BF16)       # value mask in V layout
    nc.sync.dma_start(mvb[:], io["maskVB"])

    state = [None] * HPC

    def prep_dma(h):
        st = state[h] = {}
        st["qt2"] = in2.tile([P, N], BF16, name="qt2", tag="qt2")
        nc.sync.dma_start(st["qt2"][:], io["QT2"][h])
        st["kt2"] = in2.tile([P, NT // 2, P], BF16, name="kt2", tag="kt2")
        nc.sync.dma_start(st["kt2"][:], io["KT2"][h])
        if h % 2 == 0:
            st["v2"] = v2_pool.tile([P, NT, 2, D], BF16, name="v2", tag="v2")
        else:
            st["v2"] = state[h - 1]["v2"]
        nc.sync.dma_start(
            st["v2"][:, :, h % 2, :], io["V"][h].rearrange("(t p) d -> p t d", p=P)
        )

    prep_dma(0)

    qdtr = sh.tile([P, NT, M], BF16)    # Qd^T[n, m] -> [p, t, m]
    nc.sync.dma_start(qdtr[:], io["QdTr"].rearrange("(t p) m -> p t m", p=P))
    qdnr = sh.tile([P, MT, N], BF16)    # Qd[m, q]   -> [p, c, q]
    nc.sync.dma_start(qdnr[:], io["QdNr"].rearrange("(c p) q -> p c q", p=P))

    if HPC > 1:
        prep_dma(1)

    # dummy activation: pull the exp table load off the critical path
    scr = sh.tile([P, 1], F32)
    nc.vector.memset(scr[:], 0.0)
    nc.scalar.activation(scr[:], scr[:], AF.Exp)

    def mask_inputs(h):
        st = state[h]
        nc.vector.tensor_mul(st["kt2"][:], st["kt2"][:], mkb[:])
        nc.vector.tensor_mul(
            st["v2"][:, :, h % 2, :], st["v2"][:, :, h % 2, :], mvb[:]
        )

    # ---- filler atoms (TensorE work hidden inside the act-paced spine) --
    # Small DCT matmuls run head-PAIRED: rhs [128, 2*64] spans both heads,
    # so one 128-column weight load feeds 128 streamed columns.
    def vd_atom(hp):
        """Vd = Qd @ (V*m) for head pair hp -> vd2 [128, 2, 2, 64]."""
        st = state[2 * hp]
        vd2 = st["vd2"] = vd_pool.tile([P, MT, 2, D], BF16, name="vd2", tag="vd2")
        psv = ps_m.tile([P, 512], F32, name="psv", tag="misc")
        for mh in range(MT):
            for t in range(NT):
                nc.tensor.matmul(
                    psv[:, mh * 2 * D : (mh + 1) * 2 * D],
                    lhsT=qdtr[:, t, mh * P : (mh + 1) * P],
                    rhs=st["v2"][:, t, :, :],
                    start=(t == 0),
                    stop=(t == NT - 1),
                )
        nc.vector.tensor_copy(
            vd2[:], psv[:, 0 : MT * 2 * D].rearrange("p (c h d) -> p c h d", h=2, d=D)
        )

    def u_atom(hp, quarter):
        """U = Qd^T @ Vd for 4 k-blocks of head pair hp (+ ones column)."""
        st = state[2 * hp]
        if quarter == 0:
            st["ua2"] = ua_pool.tile([P, NT, 2, D + 1], BF16, name="ua2", tag="ua2")
        ua2 = st["ua2"]
        psu = ps_m.tile([P, 512], F32, name="psu", tag="misc")
        for k4 in range(4):
            kc = quarter * 4 + k4
            for mh in range(MT):
                nc.tensor.matmul(
                    psu[:, k4 * 2 * D : (k4 + 1) * 2 * D],
                    lhsT=qdnr[:, mh, kc * P : (kc + 1) * P],
                    rhs=st["vd2"][:, mh, :, :],
                    start=(mh == 0),
                    stop=(mh == MT - 1),
                )
        nc.vector.tensor_copy(
            ua2[:, quarter * 4 : (quarter + 1) * 4, :, 0:D],
            psu[:].rearrange("p (c h d) -> p c h d", h=2, d=D),
        )
        nc.vector.memset(ua2[:, quarter * 4 : (quarter + 1) * 4, :, D : D + 1], 1.0)

    def atnu_atom(h, q):
        """One q-block of Y = atn @ [U|1]: 16 matmuls + normalize.
        Accumulator bank alternates between ps_a and ps_m so the DVE
        normalize of block q never blocks the matmuls of block q+1."""
        st = state[h]
        hp2 = h % 2
        if q == 0 and hp2 == 0:
            state[h]["au2"] = au_pool.tile([P, NT, 2, D], BF16, name="au2", tag="au2")
        if q == 0 and hp2 == 1:
            state[h]["au2"] = state[h - 1]["au2"]
        au2 = st["au2"]
        ua2 = state[h - hp2]["ua2"]
        pool = ps_a if q % 2 == 0 else ps_m
        tag = "psa" if q % 2 == 0 else "misc"
        psa = pool.tile([P, D + 1], F32, name="psa", tag=tag)
        for kc in range(NT):
            nc.tensor.matmul(
                psa[:],
                lhsT=st["ex"][:, kc, q * P : (q + 1) * P],
                rhs=ua2[:, kc, hp2, :],
                start=(kc == 0),
                stop=(kc == NT - 1),
            )
        rec = st_pool.tile([P, 1], F32, name="rec", tag="rec")
        nc.vector.reciprocal(rec[:], psa[:, D : D + 1])
        nc.vector.tensor_scalar_mul(au2[:, q, hp2, :], psa[:, 0:D], rec[:])

    def z1_atom(hp):
        """Z1 = Qd @ Y for head pair hp -> z12 [128, 2, 2, 64]."""
        st = state[2 * hp]
        z12 = st["z12"] = z1_pool.tile([P, MT, 2, D], BF16, name="z12", tag="z12")
        psz = ps_m.tile([P, 512], F32, name="psz", tag="misc")
        for mh in range(MT):
            for q in range(NT):
                nc.tensor.matmul(
                    psz[:, mh * 2 * D : (mh + 1) * 2 * D],
                    lhsT=qdtr[:, q, mh * P : (mh + 1) * P],
                    rhs=st["au2"][:, q, :, :],
                    start=(q == 0),
                    stop=(q == NT - 1),
                )
        nc.vector.tensor_copy(
            z12[:], psz[:, 0 : MT * 2 * D].rearrange("p (c h d) -> p c h d", h=2, d=D)
        )

    def out_atom(hp, quarter):
        """out = Qd^T @ Z1 for 4 q-blocks of head pair hp; DMA at the end."""
        st = state[2 * hp]
        if quarter == 0:
            st["ost2"] = ost_pool.tile([P, NT, 2, D], F32, name="ost2", tag="ost2")
        ost2 = st["ost2"]
        pso = ps_m.tile([P, 512], F32, name="pso", tag="misc")
        for q4 in range(4):
            q = quarter * 4 + q4
            for mh in range(MT):
                nc.tensor.matmul(
                    pso[:, q4 * 2 * D : (q4 + 1) * 2 * D],
                    lhsT=qdnr[:, mh, q * P : (q + 1) * P],
                    rhs=st["z12"][:, mh, :, :],
                    start=(mh == 0),
                    stop=(mh == MT - 1),
                )
        nc.vector.tensor_copy(
            ost2[:, quarter * 4 : (quarter + 1) * 4, :, :],
            pso[:].rearrange("p (c h d) -> p c h d", h=2, d=D),
        )
        if quarter == 3:
            for hh in range(2):
                nc.sync.dma_start(
                    io["out"][2 * hp + hh].rearrange("(t p) d -> p t d", p=P),
                    ost2[:, :, hh, :],
                )
            state[2 * hp] = state[2 * hp + 1] = None

    def phase_atoms(h):
        """Fillers for the S-phase of head h (indices may refer to earlier
        heads' work whose inputs completed in previous phases)."""
        atoms = []
        if h % 2 == 1 and h < HPC:
            hp = h // 2  # prep for the pair whose 2nd head phase this is
            atoms.append(lambda hp=hp: vd_atom(hp))
            for qu in range(4):
                atoms.append(lambda hp=hp, qu=qu: u_atom(hp, qu))
        if 0 <= h - 1 < HPC:
            for q in range(NT):
                atoms.append(lambda h=h, q=q: atnu_atom(h - 1, q))
        if h - 2 >= 1 and (h - 2) % 2 == 1:
            hp = (h - 2) // 2  # tail for the pair completed at end of h-1
            atoms.append(lambda hp=hp: z1_atom(hp))
            for qu in range(4):
                atoms.append(lambda hp=hp, qu=qu: out_atom(hp, qu))
        return atoms

    def s_pair_half(h, j, half):
        """Score k-blocks (2j, 2j+1) for one q-half: 4 matmuls interleaved
        across PE row groups + 2 activations."""
        st = state[h]
        pss = []
        for side in range(2):
            pss.append(ps_st.tile([P, 1024], F32, name="st", tag="st"))
        for c in range(2):
            for side in range(2):
                pr = slice(64 * side, 64 * (side + 1))
                q0 = half * 1024 + c * 512
                nc.tensor.matmul(
                    pss[side][:, c * 512 : (c + 1) * 512],
                    lhsT=st["kt2"][pr, j, :],
                    rhs=st["qt2"][pr, q0 : q0 + 512],
                    start=True,
                    stop=True,
                )
        for side in range(2):
            kc = 2 * j + side
            nc.scalar.activation(
                st["ex"][:, kc, half * 1024 : (half + 1) * 1024],
                pss[side][:],
                AF.Exp,
                scale=0.125,
            )

    # --- act-paced pipelined emission -----------------------------------
    mask_inputs(0)
    atoms = deque()
    for h in range(HPC):
        st = state[h]
        st["ex"] = exp_pool.tile([P, NT, N], BF16, name="ex", tag="ex")
        if h + 1 < HPC:
            mask_inputs(h + 1)
        atoms.extend(phase_atoms(h))
        for slot in range(NT):
            j, half = divmod(slot, 2)
            if slot == 8 and h + 2 < HPC:
                prep_dma(h + 2)
            s_pair_half(h, j, half)
            for _ in range(2):
                if atoms and (h > 0 or slot >= 2):
                    atoms.popleft()()
    for h in (HPC, HPC + 1):
        atoms.extend(phase_atoms(h))
    while atoms:
        atoms.popleft()()


def build_nc():
    from contextlib import ExitStack

    nc = bacc.Bacc("TRN2", target_bir_lowering=False, debug=False)
    io = {
        "QT2": nc.dram_tensor("QT2", [HPC, P, N], BF16, kind="ExternalInput").ap(),
        "KT2": nc.dram_tensor("KT2", [HPC, P, NT // 2, P], BF16, kind="ExternalInput").ap(),
        "V": nc.dram_tensor("V", [HPC, N, D], BF16, kind="ExternalInput").ap(),
        "maskKB": nc.dram_tensor("maskKB", [P, NT // 2, P], BF16, kind="ExternalInput").ap(),
        "maskVB": nc.dram_tensor("maskVB", [P, NT, D], BF16, kind="ExternalInput").ap(),
        "QdTr": nc.dram_tensor("QdTr", [N, M], BF16, kind="ExternalInput").ap(),
        "QdNr": nc.dram_tensor("QdNr", [M, N], BF16, kind="ExternalInput").ap(),
        "out": nc.dram_tensor("out", [HPC, N, D], F32, kind="ExternalOutput").ap(),
    }
    with tile.TileContext(nc) as tc:
        with ExitStack() as ctx:
            _emit(tc, ctx, io)
    nc.compile()
    return nc


_NC = None


def _get_nc():
    global _NC
    if _NC is None:
        _NC = build_nc()
    return _NC


def make_in_maps(Q, K, V, mask, Q_dct):
    Q = np.asarray(Q, dtype=np.float32).reshape(B * H, N, D)
    K = np.asarray(K, dtype=np.float32).reshape(B * H, N, D)
    V = np.asarray(V, dtype=np.float32).reshape(B * H, N, D)
    mask = np.asarray(mask, dtype=np.float32)
    Q_dct = np.asarray(Q_dct, dtype=np.float32)

    QT = np.ascontiguousarray(Q.transpose(0, 2, 1))          # [BH, 64, N]
    KT = np.ascontiguousarray(K.transpose(0, 2, 1))          # [BH, 64, N]
    # duplicate Q^T across both partition halves (PE row tiling)
    QT2 = np.concatenate([QT, QT], axis=1)                   # [BH, 128, N]
    # interleave K^T k-blocks: even blocks on partitions 0-63, odd on 64-127
    KTb = KT.reshape(B * H, D, NT, P)                        # [BH, 64, 16, 128]
    KT2 = np.concatenate([KTb[:, :, 0::2, :], KTb[:, :, 1::2, :]], axis=1)
    QdTr = np.ascontiguousarray(Q_dct.T).astype(NPBF16)
    QdNr = np.ascontiguousarray(Q_dct).astype(NPBF16)

    in_maps = []
    for c in range(NCORES):
        sl = slice(HPC * c, HPC * (c + 1))
        heads = list(range(HPC * c, HPC * (c + 1)))
        bs = {hp // H for hp in heads}
        assert len(bs) == 1, "all heads on a core must share a batch row"
        b = bs.pop()
        # key mask in KT2 layout [128, 8, 128] (broadcast over d-partitions)
        mk = mask[b].reshape(NT, P)                          # [16 blocks, 128]
        mkb = np.empty((P, NT // 2, P), dtype=np.float32)
        mkb[0:64] = mk[0::2][None, :, :]
        mkb[64:128] = mk[1::2][None, :, :]
        # value mask in V-tile layout [128, 16, 64]
        mvb = np.broadcast_to(mk.T[:, :, None], (P, NT, D))
        in_maps.append(
            {
                "QT2": np.ascontiguousarray(QT2[sl]).astype(NPBF16),
                "KT2": np.ascontiguousarray(KT2[sl]).astype(NPBF16),
                "V": np.ascontiguousarray(V[sl]).astype(NPBF16),
                "maskKB": np.ascontiguousarray(mkb).astype(NPBF16),
                "maskVB": np.ascontiguousarray(mvb).astype(NPBF16),
                "QdTr": QdTr,
                "QdNr": QdNr,
            }
        )
    return in_maps


def run_on_device(in_maps, **kwargs):
    nc = _get_nc()
    return bass_utils.run_bass_kernel_spmd(
        nc, in_maps, core_ids=list(range(NCORES)), **kwargs
    )


def kernel(Q, K, V, mask, Q_dct):
    in_maps = make_in_maps(Q, K, V, mask, Q_dct)
    res = run_on_device(in_maps)
    out = np.empty((B * H, N, D), dtype=np.float32)
    for c in range(NCORES):
        out[HPC * c : HPC * (c + 1)] = res.results[c]["out"]
    return out.reshape(B, H, N, D)


# revision 14
# speedup vs baseline: 1.7237x; 1.0787x over previous
"""DCT-compressed attention (nn_DCTAttentionIdeal) on 8 Trainium2 NeuronCores.

Math (per head, reference ordering):
    S    = (Q*s) @ (K*mask*s)^T with s = D**-0.25             [N,N]
    atn  = softmax(S, axis=-1)
    Vd   = Qd @ (V*mask)                                      [M,D]
    out  = Qd^T @ ((Qd @ atn @ Qd^T) @ Vd)                    [N,D]

Kernel reassociation (exact in real arithmetic):
    out = Qd^T @ (Qd @ (atn @ U)),   U = Qd^T @ (Qd @ (V*mask))  [N,D]
so the only O(N^2) contractions are the scores (N^2 D) and atn @ U
(N^2 D) -- the N^2 M path of the naive ordering is gone.

Implementation notes:
  - scores are computed TRANSPOSED (S^T[k,q]) so that exp(S^T) tiles act
    directly as matmul weights for the k-contraction of atn @ U, and the
    softmax denominator comes free as a ones-column appended to U.
  - exp is stored bf16 (fp8 mantissa is too coarse: exp quantization
    error passes straight through atn@U since Y is a random-sign sum).
  - S^T uses 2x PE row tiling (contraction D=64): even k-blocks on array
    rows 0-63, odd k-blocks on rows 64-127, chunk matmuls interleaved
    across row groups so every LDWEIGHTS hides under the other group's
    stream.
  - The kernel is paced by ScalarE's exp stream (128 activations of
    [128,1024]).  Emission is act-paced: each pair-half slot emits 4
    score matmuls + 2 activations, then pops two "filler atoms" (~8-16
    matmul slices of atn@U / DCT-tail / U-prep work for neighboring
    heads), so the strict-FIFO TensorE queue never idles on the
    act-region rotation with runnable work stuck behind it.
  - atn@U accumulators alternate between two PSUM banks so the DVE
    normalize of block q never blocks the matmuls of block q+1.
  - heads are processed in pairs for the small DCT matmuls (Vd/U/Z1/out):
    two heads' 64-wide streams share one 128-wide weight load.
  - PSUM: 6 banks = 3-deep rotation of [128,1024] score regions, 2 banks
    for the alternating atn@U accumulator / misc tail.

Sharding: batch*heads (2*16=32) split 4-per-core across 8 cores; Q_dct
replicated; no cross-core communication.  Host does layout only
(transposes / duplication for row tiling / mask broadcast layouts);
masking, softmax and all DCT algebra run on-device.
"""

from collections import deque

import numpy as np
import ml_dtypes

import concourse.tile as tile
from concourse import bacc, mybir
from concourse import bass_utils

F32 = mybir.dt.float32
BF16 = mybir.dt.bfloat16
I16 = mybir.dt.int16
ALU = mybir.AluOpType
NPBF16 = ml_dtypes.bfloat16
AF = mybir.ActivationFunctionType

B, H, N, D, M = 2, 16, 2048, 64, 256
NCORES = 8
HPC = (B * H) // NCORES  # heads per core = 4
P = 128
NT = N // P              # 16 k/q blocks
MT = M // P              # 2
SCH_A = 0.125 * 128.0 / np.log(2.0)   # Schraudolph exp: bf16 bits = A*s + B
SCH_B = 128.0 * 127.0 - 7.5


def _emit(tc, ctx, io):
    nc = tc.nc

    sh = ctx.enter_context(tc.tile_pool(name="shared", bufs=1))
    in2 = ctx.enter_context(tc.tile_pool(name="inputs", bufs=3))
    v2_pool = ctx.enter_context(tc.tile_pool(name="vpair", bufs=2))
    exp_pool = ctx.enter_context(tc.tile_pool(name="exp", bufs=2))
    ua_pool = ctx.enter_context(tc.tile_pool(name="ua", bufs=2))
    vd_pool = ctx.enter_context(tc.tile_pool(name="vd", bufs=2))
    au_pool = ctx.enter_context(tc.tile_pool(name="atnun", bufs=2))
    z1_pool = ctx.enter_context(tc.tile_pool(name="z1", bufs=2))
    ost_pool = ctx.enter_context(tc.tile_pool(name="ost", bufs=1))
    st_pool = ctx.enter_context(tc.tile_pool(name="stats", bufs=4))

    ps_st = ctx.enter_context(tc.tile_pool(name="ps_st", bufs=3, space="PSUM"))
    ps_a = ctx.enter_context(tc.tile_pool(name="ps_a", bufs=1, space="PSUM"))
    ps_m = ctx.enter_context(tc.tile_pool(name="ps_m", bufs=1, space="PSUM"))

    # --- shared, once per core (mask layouts first: first act needs them)
    mkb = sh.tile([P, NT // 2, P], BF16)  # key mask in KT2 layout
    nc.sync.dma_start(mkb[:], io["maskKB"])

    state = [None] * HPC

    def prep_dma(h):
        st = state[h] = {}
        st["qt2"] = in2.tile([P, N], BF16, name="qt2", tag="qt2")
        nc.sync.dma_start(st["qt2"][:], io["QT2"][h])
        st["kt2"] = in2.tile([P, NT // 2, P], BF16, name="kt2", tag="kt2")
        nc.sync.dma_start(st["kt2"][:], io["KT2"][h])
        if h % 2 == 0:
            st["v2"] = v2_pool.tile([P, NT, 2, D], BF16, name="v2", tag="v2")
        else:
            st["v2"] = state[h - 1]["v2"]
        nc.sync.dma_start(
            st["v2"][:, :, h % 2, :], io["V"][h].rearrange("(t p) d -> p t d", p=P)
        )

    prep_dma(0)
    mvb = sh.tile([P, NT, D], BF16)       # value mask in V layout
    nc.sync.dma_start(mvb[:], io["maskVB"])

    qdtr = sh.tile([P, NT, M], BF16)    # Qd^T[n, m] -> [p, t, m]
    nc.sync.dma_start(qdtr[:], io["QdTr"].rearrange("(t p) m -> p t m", p=P))
    qdnr = sh.tile([P, MT, N], BF16)    # Qd[m, q]   -> [p, c, q]
    nc.sync.dma_start(qdnr[:], io["QdNr"].rearrange("(c p) q -> p c q", p=P))

    if HPC > 1:
        prep_dma(1)

    # dummy activation: pull the exp table load off the critical path
    scr = sh.tile([P, 1], F32)
    nc.vector.memset(scr[:], 0.0)
    nc.scalar.activation(scr[:], scr[:], AF.Exp)

    mkbf = mkb.rearrange("p a b -> p (a b)")

    def dummy_mms(n):
        """Keep the PE's HAM activity monitor busy (K=8/8) when the pipeline
        has no real TensorE work: harmless matmuls over the mask tile."""
        psd = ps_m.tile([P, 512], F32, name="psd", tag="misc")
        for _ in range(n):
            nc.tensor.matmul(
                psd[:], lhsT=mkbf[:, 0:P], rhs=mkbf[:, 0:512], start=True, stop=True
            )

    # warm-up burst: ~7us of back-to-back matmuls during the DMA wait flips
    # the HAM clock gate to full speed before the real pipeline starts.
    dummy_mms(16)

    def mask_inputs(h):
        st = state[h]
        nc.vector.tensor_mul(st["kt2"][:], st["kt2"][:], mkb[:])
        nc.vector.tensor_mul(
            st["v2"][:, :, h % 2, :], st["v2"][:, :, h % 2, :], mvb[:]
        )

    # ---- filler atoms (TensorE work hidden inside the act-paced spine) --
    # Small DCT matmuls run head-PAIRED: rhs [128, 2*64] spans both heads,
    # so one 128-column weight load feeds 128 streamed columns.
    def vd_atom(hp):
        """Vd = Qd @ (V*m) for head pair hp -> vd2 [128, 2, 2, 64]."""
        st = state[2 * hp]
        vd2 = st["vd2"] = vd_pool.tile([P, MT, 2, D], BF16, name="vd2", tag="vd2")
        psv = ps_m.tile([P, 512], F32, name="psv", tag="misc")
        for mh in range(MT):
            for t in range(NT):
                nc.tensor.matmul(
                    psv[:, mh * 2 * D : (mh + 1) * 2 * D],
                    lhsT=qdtr[:, t, mh * P : (mh + 1) * P],
                    rhs=st["v2"][:, t, :, :],
                    start=(t == 0),
                    stop=(t == NT - 1),
                )
        nc.vector.tensor_copy(
            vd2[:], psv[:, 0 : MT * 2 * D].rearrange("p (c h d) -> p c h d", h=2, d=D)
        )

    def u_atom(hp, quarter):
        """U = Qd^T @ Vd for 4 k-blocks of head pair hp (+ ones column)."""
        st = state[2 * hp]
        if quarter == 0:
            st["ua2"] = ua_pool.tile([P, NT, 2, D + 1], BF16, name="ua2", tag="ua2")
        ua2 = st["ua2"]
        psu = ps_m.tile([P, 512], F32, name="psu", tag="misc")
        for k4 in range(4):
            kc = quarter * 4 + k4
            for mh in range(MT):
                nc.tensor.matmul(
                    psu[:, k4 * 2 * D : (k4 + 1) * 2 * D],
                    lhsT=qdnr[:, mh, kc * P : (kc + 1) * P],
                    rhs=st["vd2"][:, mh, :, :],
                    start=(mh == 0),
                    stop=(mh == MT - 1),
                )
        nc.vector.tensor_copy(
            ua2[:, quarter * 4 : (quarter + 1) * 4, :, 0:D],
            psu[:].rearrange("p (c h d) -> p c h d", h=2, d=D),
        )
        nc.vector.memset(ua2[:, quarter * 4 : (quarter + 1) * 4, :, D : D + 1], 1.0)

    def atnu_atom(h, q):
        """One q-block of Y = atn @ [U|1]: 16 matmuls + normalize.
        Accumulator bank alternates between ps_a and ps_m so the DVE
        normalize of block q never blocks the matmuls of block q+1."""
        st = state[h]
        hp2 = h % 2
        if q == 0 and hp2 == 0:
            state[h]["au2"] = au_pool.tile([P, NT, 2, D], BF16, name="au2", tag="au2")
        if q == 0 and hp2 == 1:
            state[h]["au2"] = state[h - 1]["au2"]
        au2 = st["au2"]
        ua2 = state[h - hp2]["ua2"]
        pool = ps_a if q % 2 == 0 else ps_m
        tag = "psa" if q % 2 == 0 else "misc"
        psa = pool.tile([P, D + 1], F32, name="psa", tag=tag)
        for kc in range(NT):
            nc.tensor.matmul(
                psa[:],
                lhsT=st["ex"][:, kc, q * P : (q + 1) * P],
                rhs=ua2[:, kc, hp2, :],
                start=(kc == 0),
                stop=(kc == NT - 1),
            )
        rec = st_pool.tile([P, 1], F32, name="rec", tag="rec")
        nc.vector.reciprocal(rec[:], psa[:, D : D + 1])
        nc.vector.tensor_scalar_mul(au2[:, q, hp2, :], psa[:, 0:D], rec[:])

    def z1_atom(hp):
        """Z1 = Qd @ Y for head pair hp -> z12 [128, 2, 2, 64]."""
        st = state[2 * hp]
        z12 = st["z12"] = z1_pool.tile([P, MT, 2, D], BF16, name="z12", tag="z12")
        psz = ps_m.tile([P, 512], F32, name="psz", tag="misc")
        for mh in range(MT):
            for q in range(NT):
                nc.tensor.matmul(
                    psz[:, mh * 2 * D : (mh + 1) * 2 * D],
                    lhsT=qdtr[:, q, mh * P : (mh + 1) * P],
                    rhs=st["au2"][:, q, :, :],
                    start=(q == 0),
                    stop=(q == NT - 1),
                )
        nc.vector.tensor_copy(
            z12[:], psz[:, 0 : MT * 2 * D].rearrange("p (c h d) -> p c h d", h=2, d=D)
        )

    def out_atom(hp, quarter):
        """out = Qd^T @ Z1 for 4 q-blocks of head pair hp; DMA at the end."""
        st = state[2 * hp]
        if quarter == 0:
            st["ost2"] = ost_pool.tile([P, NT, 2, D], F32, name="ost2", tag="ost2")
        ost2 = st["ost2"]
        pso = ps_m.tile([P, 512], F32, name="pso", tag="misc")
        for q4 in range(4):
            q = quarter * 4 + q4
            for mh in range(MT):
                nc.tensor.matmul(
                    pso[:, q4 * 2 * D : (q4 + 1) * 2 * D],
                    lhsT=qdnr[:, mh, q * P : (q + 1) * P],
                    rhs=st["z12"][:, mh, :, :],
                    start=(mh == 0),
                    stop=(mh == MT - 1),
                )
        nc.vector.tensor_copy(
            ost2[:, quarter * 4 : (quarter + 1) * 4, :, :],
            pso[:].rearrange("p (c h d) -> p c h d", h=2, d=D),
        )
        if quarter == 3:
            for hh in range(2):
                nc.sync.dma_start(
                    io["out"][2 * hp + hh].rearrange("(t p) d -> p t d", p=P),
                    ost2[:, :, hh, :],
                )
            state[2 * hp] = state[2 * hp + 1] = None

    def phase_atoms(h):
        """Fillers for the S-phase of head h (indices may refer to earlier
        heads' work whose inputs completed in previous phases).  Dummy
        matmuls pad phases whose real TensorE work is far below the act
        spine, so the HAM clock gate never re-throttles the PE."""
        atoms = []
        prep = []
        if h % 2 == 1 and h < HPC:
            hp = h // 2  # prep for the pair whose 2nd head phase this is
            prep.append(lambda hp=hp: vd_atom(hp))
            for qu in range(4):
                prep.append(lambda hp=hp, qu=qu: u_atom(hp, qu))
        work = []
        if 0 <= h - 1 < HPC:
            for q in range(NT):
                work.append(lambda h=h, q=q: atnu_atom(h - 1, q))
        tail = []
        if h - 2 >= 1 and (h - 2) % 2 == 1:
            hp = (h - 2) // 2  # tail for the pair completed at end of h-1
            tail.append(lambda hp=hp: z1_atom(hp))
            for qu in range(4):
                tail.append(lambda hp=hp, qu=qu: out_atom(hp, qu))
        n_dum = {0: 10, 1: 4}.get(h, 0)
        atoms = prep + work + tail
        # interleave dummies after every 2nd real atom
        if n_dum:
            out = []
            k = 0
            for a in atoms:
                out.append(a)
                k += 1
                if k % 2 == 0 and n_dum > 0:
                    out.append(lambda: dummy_mms(4))
                    n_dum -= 1
            out.extend([lambda: dummy_mms(4)] * n_dum)
            atoms = out
        return atoms

    def s_pair_half(h, j, half):
        """Score k-blocks (2j, 2j+1) for one q-half: 4 matmuls interleaved
        across PE row groups + 2 exp evaluations.  In phases 0 and HPC-1
        (which lack filler work / gate the epilogue) some units compute exp
        on the DVE instead via the Schraudolph bit trick: bf16 bits =
        round(A*scores + B), evaluated as one tensor_scalar into an int16
        view of the exp tile (rel err ~1.8%, cancels partly in softmax)."""
        st = state[h]
        slot = 2 * j + half
        sch = h in (0, HPC - 1) and 2 <= slot <= 13
        pss = []
        for side in range(2):
            pss.append(ps_st.tile([P, 1024], F32, name="st", tag="st"))
        for c in range(2):
            for side in range(2):
                pr = slice(64 * side, 64 * (side + 1))
                q0 = half * 1024 + c * 512
                nc.tensor.matmul(
                    pss[side][:, c * 512 : (c + 1) * 512],
                    lhsT=st["kt2"][pr, j, :],
                    rhs=st["qt2"][pr, q0 : q0 + 512],
                    start=True,
                    stop=True,
                )
        for side in range(2):
            kc = 2 * j + side
            dst = st["ex"][:, kc, half * 1024 : (half + 1) * 1024]
            if sch and side == 1:
                nc.vector.tensor_scalar(
                    dst.bitcast(I16), pss[side][:], SCH_A, SCH_B,
                    op0=ALU.mult, op1=ALU.add,
                )
            else:
                nc.scalar.activation(dst, pss[side][:], AF.Exp, scale=0.125)

    # --- act-paced pipelined emission -----------------------------------
    mask_inputs(0)
    atoms = deque()
    for h in range(HPC):
        st = state[h]
        st["ex"] = exp_pool.tile([P, NT, N], BF16, name="ex", tag="ex")
        if h + 1 < HPC:
            mask_inputs(h + 1)
        atoms.extend(phase_atoms(h))
        for slot in range(NT):
            j, half = divmod(slot, 2)
            if slot == 8 and h + 2 < HPC:
                prep_dma(h + 2)
            s_pair_half(h, j, half)
            for _ in range(2):
                if atoms and (h > 0 or slot >= 2):
                    atoms.popleft()()
    for h in (HPC, HPC + 1):
        atoms.extend(phase_atoms(h))
    while atoms:
        atoms.popleft()()


def build_nc():
    from contextlib import ExitStack

    nc = bacc.Bacc("TRN2", target_bir_lowering=False, debug=False)
    io = {
        "QT2": nc.dram_tensor("QT2", [HPC, P, N], BF16, kind="ExternalInput").ap(),
        "KT2": nc.dram_tensor("KT2", [HPC, P, NT // 2, P], BF16, kind="ExternalInput").ap(),
        "V": nc.dram_tensor("V", [HPC, N, D], BF16, kind="ExternalInput").ap(),
        "maskKB": nc.dram_tensor("maskKB", [P, NT // 2, P], BF16, kind="ExternalInput").ap(),
        "maskVB": nc.dram_tensor("maskVB", [P, NT, D], BF16, kind="ExternalInput").ap(),
        "QdTr": nc.dram_tensor("QdTr", [N, M], BF16, kind="ExternalInput").ap(),
        "QdNr": nc.dram_tensor("QdNr", [M, N], BF16, kind="ExternalInput").ap(),
        "out": nc.dram_tensor("out", [HPC, N, D], F32, kind="ExternalOutput").ap(),
    }
    with tile.TileContext(nc) as tc:
        with ExitStack() as ctx:
            _emit(tc, ctx, io)
    nc.compile()
    return nc


_NC = None


def _get_nc():
    global _NC
    if _NC is None:
        _NC = build_nc()
    return _NC


def make_in_maps(Q, K, V, mask, Q_dct):
    Q = np.asarray(Q, dtype=np.float32).reshape(B * H, N, D)
    K = np.asarray(K, dtype=np.float32).reshape(B * H, N, D)
    V = np.asarray(V, dtype=np.float32).reshape(B * H, N, D)
    mask = np.asarray(mask, dtype=np.float32)
    Q_dct = np.asarray(Q_dct, dtype=np.float32)

    QT = np.ascontiguousarray(Q.transpose(0, 2, 1))          # [BH, 64, N]
    KT = np.ascontiguousarray(K.transpose(0, 2, 1))          # [BH, 64, N]
    # duplicate Q^T across both partition halves (PE row tiling)
    QT2 = np.concatenate([QT, QT], axis=1)                   # [BH, 128, N]
    # interleave K^T k-blocks: even blocks on partitions 0-63, odd on 64-127
    KTb = KT.reshape(B * H, D, NT, P)                        # [BH, 64, 16, 128]
    KT2 = np.concatenate([KTb[:, :, 0::2, :], KTb[:, :, 1::2, :]], axis=1)
    QdTr = np.ascontiguousarray(Q_dct.T).astype(NPBF16)
    QdNr = np.ascontiguousarray(Q_dct).astype(NPBF16)

    in_maps = []
    for c in range(NCORES):
        sl = slice(HPC * c, HPC * (c + 1))
        heads = list(range(HPC * c, HPC * (c + 1)))
        bs = {hp // H for hp in heads}
        assert len(bs) == 1, "all heads on a core must share a batch row"
        b = bs.pop()
        # key mask in KT2 layout [128, 8, 128] (broadcast over d-partitions)
        mk = mask[b].reshape(NT, P)                          # [16 blocks, 128]
        mkb = np.empty((P, NT // 2, P), dtype=np.float32)
        mkb[0:64] = mk[0::2][None, :, :]
        mkb[64:128] = mk[1::2][None, :, :]
        # value mask in V-tile layout [128, 16, 64]
        mvb = np.broadcast_to(mk.T[:, :, None], (P, NT, D))
        in_maps.append(
            {
                "QT2": np.ascontiguousarray(QT2[sl]).astype(NPBF16),
                "KT2": np.ascontiguousarray(KT2[sl]).astype(NPBF16),
                "V": np.ascontiguousarray(V[sl]).astype(NPBF16),
                "maskKB": np.ascontiguousarray(mkb).astype(NPBF16),
                "maskVB": np.ascontiguousarray(mvb).astype(NPBF16),
                "QdTr": QdTr,
                "QdNr": QdNr,
            }
        )
    return in_maps


def run_on_device(in_maps, **kwargs):
    nc = _get_nc()
    return bass_utils.run_bass_kernel_spmd(
        nc, in_maps, core_ids=list(range(NCORES)), **kwargs
    )


def kernel(Q, K, V, mask, Q_dct):
    in_maps = make_in_maps(Q, K, V, mask, Q_dct)
    res = run_on_device(in_maps)
    out = np.empty((B * H, N, D), dtype=np.float32)
    for c in range(NCORES):
        out[HPC * c : HPC * (c + 1)] = res.results[c]["out"]
    return out.reshape(B, H, N, D)


# revision 15
# speedup vs baseline: 1.9595x; 1.1368x over previous
"""DCT-compressed attention (nn_DCTAttentionIdeal) on 8 Trainium2 NeuronCores.

Math (per head, reference ordering):
    S    = (Q*s) @ (K*mask*s)^T with s = D**-0.25             [N,N]
    atn  = softmax(S, axis=-1)
    Vd   = Qd @ (V*mask)                                      [M,D]
    out  = Qd^T @ ((Qd @ atn @ Qd^T) @ Vd)                    [N,D]

Kernel reassociation (exact in real arithmetic):
    out = Qd^T @ (Qd @ (atn @ U)),   U = Qd^T @ (Qd @ (V*mask))  [N,D]
so the only O(N^2) contractions are the scores (N^2 D) and atn @ U
(N^2 D) -- the N^2 M path of the naive ordering is gone.

Implementation notes:
  - scores are computed TRANSPOSED (S^T[k,q]) so that exp(S^T) tiles act
    directly as matmul weights for the k-contraction of atn @ U, and the
    softmax denominator comes free as a ones-column appended to U.
  - exp is stored bf16 (fp8 mantissa is too coarse: exp quantization
    error passes straight through atn@U since Y is a random-sign sum).
  - S^T uses 2x PE row tiling (contraction D=64): even k-blocks on array
    rows 0-63, odd k-blocks on rows 64-127, chunk matmuls interleaved
    across row groups so every LDWEIGHTS hides under the other group's
    stream.
  - The kernel is paced by ScalarE's exp stream (128 activations of
    [128,1024]).  Emission is act-paced: each pair-half slot emits 4
    score matmuls + 2 activations, then pops two "filler atoms" (~8-16
    matmul slices of atn@U / DCT-tail / U-prep work for neighboring
    heads), so the strict-FIFO TensorE queue never idles on the
    act-region rotation with runnable work stuck behind it.
  - atn@U accumulators alternate between two PSUM banks so the DVE
    normalize of block q never blocks the matmuls of block q+1.
  - heads are processed in pairs for the small DCT matmuls (Vd/U/Z1/out):
    two heads' 64-wide streams share one 128-wide weight load.
  - PSUM: 6 banks = 3-deep rotation of [128,1024] score regions, 2 banks
    for the alternating atn@U accumulator / misc tail.

Sharding: batch*heads (2*16=32) split 4-per-core across 8 cores; Q_dct
replicated; no cross-core communication.  Host does layout only
(transposes / duplication for row tiling / mask broadcast layouts);
masking, softmax and all DCT algebra run on-device.
"""

from collections import deque

import numpy as np
import ml_dtypes

import concourse.tile as tile
from concourse import bacc, mybir
from concourse import bass_utils

F32 = mybir.dt.float32
BF16 = mybir.dt.bfloat16
I16 = mybir.dt.int16
ALU = mybir.AluOpType
NPBF16 = ml_dtypes.bfloat16
AF = mybir.ActivationFunctionType

B, H, N, D, M = 2, 16, 2048, 64, 256
NCORES = 8
HPC = (B * H) // NCORES  # heads per core = 4
P = 128
NT = N // P              # 16 k/q blocks
MT = M // P              # 2
SCH_A = 0.125 * 128.0 / np.log(2.0)   # Schraudolph exp: bf16 bits = A*s + B
SCH_B = 128.0 * 127.0 - 7.5


def _emit(tc, ctx, io):
    nc = tc.nc

    sh = ctx.enter_context(tc.tile_pool(name="shared", bufs=1))
    in2 = ctx.enter_context(tc.tile_pool(name="inputs", bufs=3))
    v2_pool = ctx.enter_context(tc.tile_pool(name="vpair", bufs=2))
    exp_pool = ctx.enter_context(tc.tile_pool(name="exp", bufs=2))
    ua_pool = ctx.enter_context(tc.tile_pool(name="ua", bufs=2))
    vd_pool = ctx.enter_context(tc.tile_pool(name="vd", bufs=2))
    au_pool = ctx.enter_context(tc.tile_pool(name="atnun", bufs=2))
    z1_pool = ctx.enter_context(tc.tile_pool(name="z1", bufs=2))
    ost_pool = ctx.enter_context(tc.tile_pool(name="ost", bufs=1))
    st_pool = ctx.enter_context(tc.tile_pool(name="stats", bufs=4))

    ps_st = ctx.enter_context(tc.tile_pool(name="ps_st", bufs=3, space="PSUM"))
    ps_a = ctx.enter_context(tc.tile_pool(name="ps_a", bufs=1, space="PSUM"))
    ps_m = ctx.enter_context(tc.tile_pool(name="ps_m", bufs=1, space="PSUM"))

    # --- shared, once per core (mask layouts first: first act needs them)
    mkb = sh.tile([P, NT // 2, P], BF16)  # key mask in KT2 layout
    nc.sync.dma_start(mkb[:], io["maskKB"])

    state = [None] * HPC

    def prep_dma(h):
        st = state[h] = {}
        st["qt2"] = in2.tile([P, N], BF16, name="qt2", tag="qt2")
        nc.sync.dma_start(st["qt2"][:], io["QT2"][h])
        st["kt2"] = in2.tile([P, NT // 2, P], BF16, name="kt2", tag="kt2")
        nc.sync.dma_start(st["kt2"][:], io["KT2"][h])
        if h % 2 == 0:
            st["v2"] = v2_pool.tile([P, NT, 2, D], BF16, name="v2", tag="v2")
        else:
            st["v2"] = state[h - 1]["v2"]
        nc.sync.dma_start(
            st["v2"][:, :, h % 2, :], io["V"][h].rearrange("(t p) d -> p t d", p=P)
        )

    prep_dma(0)
    mvb = sh.tile([P, NT, D], BF16)       # value mask in V layout
    nc.sync.dma_start(mvb[:], io["maskVB"])

    qdtr = sh.tile([P, NT, M], BF16)    # Qd^T[n, m] -> [p, t, m]
    nc.sync.dma_start(qdtr[:], io["QdTr"].rearrange("(t p) m -> p t m", p=P))
    qdnr = sh.tile([P, MT, N], BF16)    # Qd[m, q]   -> [p, c, q]
    nc.sync.dma_start(qdnr[:], io["QdNr"].rearrange("(c p) q -> p c q", p=P))

    if HPC > 1:
        prep_dma(1)

    # dummy activation: pull the exp table load off the critical path
    scr = sh.tile([P, 1], F32)
    nc.vector.memset(scr[:], 0.0)
    nc.scalar.activation(scr[:], scr[:], AF.Exp)

    mkbf = mkb.rearrange("p a b -> p (a b)")

    def dummy_mms(n):
        """Keep the PE's HAM activity monitor busy (K=8/8) when the pipeline
        has no real TensorE work: harmless matmuls over the mask tile."""
        psd = ps_m.tile([P, 512], F32, name="psd", tag="misc")
        for _ in range(n):
            nc.tensor.matmul(
                psd[:], lhsT=mkbf[:, 0:P], rhs=mkbf[:, 0:512], start=True, stop=True
            )

    # warm-up burst: ~7us of back-to-back matmuls during the DMA wait flips
    # the HAM clock gate to full speed before the real pipeline starts.
    dummy_mms(16)

    def mask_inputs(h):
        st = state[h]
        nc.vector.tensor_mul(st["kt2"][:], st["kt2"][:], mkb[:])
        nc.vector.tensor_mul(
            st["v2"][:, :, h % 2, :], st["v2"][:, :, h % 2, :], mvb[:]
        )

    # ---- filler atoms (TensorE work hidden inside the act-paced spine) --
    # Small DCT matmuls run head-PAIRED: rhs [128, 2*64] spans both heads,
    # so one 128-column weight load feeds 128 streamed columns.
    def vd_atom(hp):
        """Vd = Qd @ (V*m) for head pair hp -> vd2 [128, 2, 2, 64]."""
        st = state[2 * hp]
        vd2 = st["vd2"] = vd_pool.tile([P, MT, 2, D], BF16, name="vd2", tag="vd2")
        psv = ps_m.tile([P, 512], F32, name="psv", tag="misc")
        for mh in range(MT):
            for t in range(NT):
                nc.tensor.matmul(
                    psv[:, mh * 2 * D : (mh + 1) * 2 * D],
                    lhsT=qdtr[:, t, mh * P : (mh + 1) * P],
                    rhs=st["v2"][:, t, :, :],
                    start=(t == 0),
                    stop=(t == NT - 1),
                )
        nc.vector.tensor_copy(
            vd2[:], psv[:, 0 : MT * 2 * D].rearrange("p (c h d) -> p c h d", h=2, d=D)
        )

    def u_atom(hp, quarter):
        """U = Qd^T @ Vd for 4 k-blocks of head pair hp (+ ones column)."""
        st = state[2 * hp]
        if quarter == 0:
            st["ua2"] = ua_pool.tile([P, NT, 2, D + 1], BF16, name="ua2", tag="ua2")
        ua2 = st["ua2"]
        psu = ps_m.tile([P, 512], F32, name="psu", tag="misc")
        for k4 in range(4):
            kc = quarter * 4 + k4
            for mh in range(MT):
                nc.tensor.matmul(
                    psu[:, k4 * 2 * D : (k4 + 1) * 2 * D],
                    lhsT=qdnr[:, mh, kc * P : (kc + 1) * P],
                    rhs=st["vd2"][:, mh, :, :],
                    start=(mh == 0),
                    stop=(mh == MT - 1),
                )
        nc.vector.tensor_copy(
            ua2[:, quarter * 4 : (quarter + 1) * 4, :, 0:D],
            psu[:].rearrange("p (c h d) -> p c h d", h=2, d=D),
        )
        nc.vector.memset(ua2[:, quarter * 4 : (quarter + 1) * 4, :, D : D + 1], 1.0)

    def atnu_atom(h, q):
        """One q-block of Y = atn @ [U|1]: 16 matmuls + normalize.
        Accumulator bank alternates between ps_a and ps_m so the DVE
        normalize of block q never blocks the matmuls of block q+1."""
        st = state[h]
        hp2 = h % 2
        if q == 0 and hp2 == 0:
            state[h]["au2"] = au_pool.tile([P, NT, 2, D], BF16, name="au2", tag="au2")
        if q == 0 and hp2 == 1:
            state[h]["au2"] = state[h - 1]["au2"]
        au2 = st["au2"]
        ua2 = state[h - hp2]["ua2"]
        pool = ps_a if q % 2 == 0 else ps_m
        tag = "psa" if q % 2 == 0 else "misc"
        psa = pool.tile([P, D + 1], F32, name="psa", tag=tag)
        for kc in range(NT):
            nc.tensor.matmul(
                psa[:],
                lhsT=st["ex"][:, kc, q * P : (q + 1) * P],
                rhs=ua2[:, kc, hp2, :],
                start=(kc == 0),
                stop=(kc == NT - 1),
            )
        rec = st_pool.tile([P, 1], F32, name="rec", tag="rec")
        nc.vector.reciprocal(rec[:], psa[:, D : D + 1])
        nc.vector.tensor_scalar_mul(au2[:, q, hp2, :], psa[:, 0:D], rec[:])

    def z1_atom(hp):
        """Z1 = Qd @ Y for head pair hp -> z12 [128, 2, 2, 64]."""
        st = state[2 * hp]
        z12 = st["z12"] = z1_pool.tile([P, MT, 2, D], BF16, name="z12", tag="z12")
        psz = ps_m.tile([P, 512], F32, name="psz", tag="misc")
        for mh in range(MT):
            for q in range(NT):
                nc.tensor.matmul(
                    psz[:, mh * 2 * D : (mh + 1) * 2 * D],
                    lhsT=qdtr[:, q, mh * P : (mh + 1) * P],
                    rhs=st["au2"][:, q, :, :],
                    start=(q == 0),
                    stop=(q == NT - 1),
                )
        nc.vector.tensor_copy(
            z12[:], psz[:, 0 : MT * 2 * D].rearrange("p (c h d) -> p c h d", h=2, d=D)
        )

    def out_atom(hp, quarter):
        """out = Qd^T @ Z1 for 4 q-blocks of head pair hp; DMA at the end."""
        st = state[2 * hp]
        if quarter == 0:
            st["ost2"] = ost_pool.tile([P, NT, 2, D], F32, name="ost2", tag="ost2")
        ost2 = st["ost2"]
        pso = ps_m.tile([P, 512], F32, name="pso", tag="misc")
        for q4 in range(4):
            q = quarter * 4 + q4
            for mh in range(MT):
                nc.tensor.matmul(
                    pso[:, q4 * 2 * D : (q4 + 1) * 2 * D],
                    lhsT=qdnr[:, mh, q * P : (q + 1) * P],
                    rhs=st["z12"][:, mh, :, :],
                    start=(mh == 0),
                    stop=(mh == MT - 1),
                )
        nc.vector.tensor_copy(
            ost2[:, quarter * 4 : (quarter + 1) * 4, :, :],
            pso[:].rearrange("p (c h d) -> p c h d", h=2, d=D),
        )
        if quarter == 3:
            for hh in range(2):
                nc.sync.dma_start(
                    io["out"][2 * hp + hh].rearrange("(t p) d -> p t d", p=P),
                    ost2[:, :, hh, :],
                )
            state[2 * hp] = state[2 * hp + 1] = None

    def phase_atoms(h):
        """Fillers for the S-phase of head h (indices may refer to earlier
        heads' work whose inputs completed in previous phases).  Dummy
        matmuls pad phases whose real TensorE work is far below the act
        spine, so the HAM clock gate never re-throttles the PE."""
        atoms = []
        prep = []
        if h % 2 == 1 and h < HPC:
            hp = h // 2  # prep for the pair whose 2nd head phase this is
            prep.append(lambda hp=hp: vd_atom(hp))
            for qu in range(4):
                prep.append(lambda hp=hp, qu=qu: u_atom(hp, qu))
        work = []
        if 0 <= h - 1 < HPC:
            for q in range(NT):
                work.append(lambda h=h, q=q: atnu_atom(h - 1, q))
        tail = []
        if h - 2 >= 1 and (h - 2) % 2 == 1:
            hp = (h - 2) // 2  # tail for the pair completed at end of h-1
            tail.append(lambda hp=hp: z1_atom(hp))
            for qu in range(4):
                tail.append(lambda hp=hp, qu=qu: out_atom(hp, qu))
        n_dum = {0: 10, 1: 2}.get(h, 0)
        atoms = prep + work + tail
        # interleave dummies after every 2nd real atom
        if n_dum:
            out = []
            k = 0
            for a in atoms:
                out.append(a)
                k += 1
                if k % 2 == 0 and n_dum > 0:
                    out.append(lambda: dummy_mms(4))
                    n_dum -= 1
            out.extend([lambda: dummy_mms(4)] * n_dum)
            atoms = out
        return atoms

    def s_pair_half(h, j, half):
        """Score k-blocks (2j, 2j+1) for one q-half: 4 matmuls interleaved
        across PE row groups + 2 exp evaluations.  In phases 0 and HPC-1
        (which lack filler work / gate the epilogue) some units compute exp
        on the DVE instead via the Schraudolph bit trick: bf16 bits =
        round(A*scores + B), evaluated as one tensor_scalar into an int16
        view of the exp tile (rel err ~1.8%, cancels partly in softmax)."""
        st = state[h]
        slot = 2 * j + half
        if h in (0, HPC - 1):
            sch = 2 <= slot <= 13
        else:
            sch = 8 <= slot <= 13
        pss = []
        for side in range(2):
            pss.append(ps_st.tile([P, 1024], F32, name="st", tag="st"))
        for c in range(2):
            for side in range(2):
                pr = slice(64 * side, 64 * (side + 1))
                q0 = half * 1024 + c * 512
                nc.tensor.matmul(
                    pss[side][:, c * 512 : (c + 1) * 512],
                    lhsT=st["kt2"][pr, j, :],
                    rhs=st["qt2"][pr, q0 : q0 + 512],
                    start=True,
                    stop=True,
                )
        for side in range(2):
            kc = 2 * j + side
            dst = st["ex"][:, kc, half * 1024 : (half + 1) * 1024]
            if sch and side == 1:
                nc.vector.tensor_scalar(
                    dst.bitcast(I16), pss[side][:], SCH_A, SCH_B,
                    op0=ALU.mult, op1=ALU.add,
                )
            else:
                nc.scalar.activation(dst, pss[side][:], AF.Exp, scale=0.125)

    # --- act-paced pipelined emission -----------------------------------
    mask_inputs(0)
    atoms = deque()
    for h in range(HPC):
        st = state[h]
        st["ex"] = exp_pool.tile([P, NT, N], BF16, name="ex", tag="ex")
        if h + 1 < HPC:
            mask_inputs(h + 1)
        atoms.extend(phase_atoms(h))
        for slot in range(NT):
            j, half = divmod(slot, 2)
            if slot == 8 and h + 2 < HPC:
                prep_dma(h + 2)
            s_pair_half(h, j, half)
            if h > 0 or slot >= 2:
                want = (len(atoms) + (NT - 1 - slot)) // (NT - slot)
                for _ in range(min(want, 3)):
                    if atoms:
                        atoms.popleft()()
    for h in (HPC, HPC + 1):
        atoms.extend(phase_atoms(h))
    while atoms:
        atoms.popleft()()


def build_nc():
    from contextlib import ExitStack

    nc = bacc.Bacc("TRN2", target_bir_lowering=False, debug=False)
    io = {
        "QT2": nc.dram_tensor("QT2", [HPC, P, N], BF16, kind="ExternalInput").ap(),
        "KT2": nc.dram_tensor("KT2", [HPC, P, NT // 2, P], BF16, kind="ExternalInput").ap(),
        "V": nc.dram_tensor("V", [HPC, N, D], BF16, kind="ExternalInput").ap(),
        "maskKB": nc.dram_tensor("maskKB", [P, NT // 2, P], BF16, kind="ExternalInput").ap(),
        "maskVB": nc.dram_tensor("maskVB", [P, NT, D], BF16, kind="ExternalInput").ap(),
        "QdTr": nc.dram_tensor("QdTr", [N, M], BF16, kind="ExternalInput").ap(),
        "QdNr": nc.dram_tensor("QdNr", [M, N], BF16, kind="ExternalInput").ap(),
        "out": nc.dram_tensor("out", [HPC, N, D], F32, kind="ExternalOutput").ap(),
    }
    with tile.TileContext(nc) as tc:
        with ExitStack() as ctx:
            _emit(tc, ctx, io)
    nc.compile()
    return nc


_NC = None


def _get_nc():
    global _NC
    if _NC is None:
        _NC = build_nc()
    return _NC


def make_in_maps(Q, K, V, mask, Q_dct):
    Q = np.asarray(Q, dtype=np.float32).reshape(B * H, N, D)
    K = np.asarray(K, dtype=np.float32).reshape(B * H, N, D)
    V = np.asarray(V, dtype=np.float32).reshape(B * H, N, D)
    mask = np.asarray(mask, dtype=np.float32)
    Q_dct = np.asarray(Q_dct, dtype=np.float32)

    QT = np.ascontiguousarray(Q.transpose(0, 2, 1))          # [BH, 64, N]
    KT = np.ascontiguousarray(K.transpose(0, 2, 1))          # [BH, 64, N]
    # duplicate Q^T across both partition halves (PE row tiling)
    QT2 = np.concatenate([QT, QT], axis=1)                   # [BH, 128, N]
    # interleave K^T k-blocks: even blocks on partitions 0-63, odd on 64-127
    KTb = KT.reshape(B * H, D, NT, P)                        # [BH, 64, 16, 128]
    KT2 = np.concatenate([KTb[:, :, 0::2, :], KTb[:, :, 1::2, :]], axis=1)
    QdTr = np.ascontiguousarray(Q_dct.T).astype(NPBF16)
    QdNr = np.ascontiguousarray(Q_dct).astype(NPBF16)

    in_maps = []
    for c in range(NCORES):
        sl = slice(HPC * c, HPC * (c + 1))
        heads = list(range(HPC * c, HPC * (c + 1)))
        bs = {hp // H for hp in heads}
        assert len(bs) == 1, "all heads on a core must share a batch row"
        b = bs.pop()
        # key mask in KT2 layout [128, 8, 128] (broadcast over d-partitions)
        mk = mask[b].reshape(NT, P)                          # [16 blocks, 128]
        mkb = np.empty((P, NT // 2, P), dtype=np.float32)
        mkb[0:64] = mk[0::2][None, :, :]
        mkb[64:128] = mk[1::2][None, :, :]
        # value mask in V-tile layout [128, 16, 64]
        mvb = np.broadcast_to(mk.T[:, :, None], (P, NT, D))
        in_maps.append(
            {
                "QT2": np.ascontiguousarray(QT2[sl]).astype(NPBF16),
                "KT2": np.ascontiguousarray(KT2[sl]).astype(NPBF16),
                "V": np.ascontiguousarray(V[sl]).astype(NPBF16),
                "maskKB": np.ascontiguousarray(mkb).astype(NPBF16),
                "maskVB": np.ascontiguousarray(mvb).astype(NPBF16),
                "QdTr": QdTr,
                "QdNr": QdNr,
            }
        )
    return in_maps


def run_on_device(in_maps, **kwargs):
    nc = _get_nc()
    return bass_utils.run_bass_kernel_spmd(
        nc, in_maps, core_ids=list(range(NCORES)), **kwargs
    )


def kernel(Q, K, V, mask, Q_dct):
    in_maps = make_in_maps(Q, K, V, mask, Q_dct)
    res = run_on_device(in_maps)
    out = np.empty((B * H, N, D), dtype=np.float32)
    for c in range(NCORES):
        out[HPC * c : HPC * (c + 1)] = res.results[c]["out"]
    return out.reshape(B, H, N, D)
